# revision 9
# baseline (speedup 1.0000x reference)
"""Llama forward-pass Trainium2 kernel: 8-core tensor-parallel (column-sharded
weights + AllGather between GEMM groups), bf16 matmuls with fp32 accumulation.

Self-contained: hardcodes all shapes. kernel(**inputs) -> logits [2,1024,32000].
"""
import numpy as np
import ml_dtypes

import concourse.bass as bass
import concourse.bacc as bacc
import concourse.mybir as mybir
import concourse.tile as tile
from concourse.bass import ts
from concourse.bass_utils import run_bass_kernel_spmd
from concourse.masks import make_identity

AF = mybir.ActivationFunctionType
OP = mybir.AluOpType
BF16 = mybir.dt.bfloat16
F32 = mybir.dt.float32
nbf16 = ml_dtypes.bfloat16

NCORES = 8
L = 4
DIM = 2048
KD = DIM // 128            # 16
HEADS = 32
HD = 64
QH = 4                     # q heads per core
B, S = 2, 1024
M = B * S                  # 2048 tokens
NMT = 4                    # m tiles of 512
MT = 512
HIDDEN = 5632
FFN_N = 704                # per-core ffn cols
FFN_PAD = 768              # padded to 6 x 128
HID_PAD = FFN_PAD * NCORES # 6144
W2KD = HID_PAD // 128      # 48
OUT_N = 4000
OUT_PAD = 4096
NF = OUT_PAD // 128        # 32
EPS = 1e-5
RG = [list(range(NCORES))]


def _build():
    nc = bacc.Bacc("TRN2", target_bir_lowering=False, debug=False,
                   num_devices=NCORES)

    hn0_d = nc.dram_tensor("hn0", [128, KD, M], BF16, kind="ExternalInput")
    h0r_d = nc.dram_tensor("h0r", [128, 2, M], F32, kind="ExternalInput")
    cos_d = nc.dram_tensor("cosst", [128, M], BF16, kind="ExternalInput")
    mask_d = nc.dram_tensor("maskA", [128, 128], F32, kind="ExternalInput")
    wq_d = nc.dram_tensor("wq", [L, 128, 2, KD, 128], BF16, kind="ExternalInput")
    wkv_d = nc.dram_tensor("wkv", [L, 128, KD, 128], BF16, kind="ExternalInput")
    wo_d = nc.dram_tensor("wo", [L, 128, KD, 256], BF16, kind="ExternalInput")
    w1_d = nc.dram_tensor("w1", [L, 128, 6, KD, 128], BF16, kind="ExternalInput")
    w3_d = nc.dram_tensor("w3", [L, 128, 6, KD, 128], BF16, kind="ExternalInput")
    w2_d = nc.dram_tensor("w2", [L, 128, W2KD, 256], BF16, kind="ExternalInput")
    wout_d = nc.dram_tensor("wout", [128, NF, KD, 128], BF16, kind="ExternalInput")
    logT_d = nc.dram_tensor("logT", [OUT_PAD, M], F32, kind="ExternalOutput")

    with tile.TileContext(nc) as tc:
        with (
            tc.tile_pool(name="cst", bufs=1) as cst,
            tc.tile_pool(name="sb", bufs=2) as sb,
            tc.tile_pool(name="ps", bufs=8, space="PSUM") as ps,
            tc.tile_pool(name="dd", bufs=2, space="DRAM") as dd,
        ):
            # ---- constants ----
            cosst = cst.tile([128, M], BF16)
            nc.sync.dma_start(cosst[:], cos_d[:])
            maskA = cst.tile([128, 128], F32)
            nc.sync.dma_start(maskA[:], mask_d[:])
            ident = cst.tile([128, 128], BF16)
            make_identity(nc, ident[:])
            ones_sq = cst.tile([128, 1], BF16)
            nc.vector.memset(ones_sq[:], 1.0)
            ones_bc = cst.tile([1, 128], F32)
            nc.vector.memset(ones_bc[:], 1.0)
            eps_t = cst.tile([1, 1], F32)
            nc.vector.memset(eps_t[:], EPS)

            # ---- persistent activations ----
            xn = sb.tile([128, KD, M], BF16, tag="xn", bufs=1)
            cur_h = sb.tile([128, 2, M], F32, tag="h2", bufs=1)
            nc.sync.dma_start(cur_h[:], h0r_d[:])

            def norm_pass(src_ap):
                """src_ap: [128, KD, M] bf16 view. Writes xn = src * rsqrt(mean sq)."""
                for mi in range(NMT):
                    hn_t = sb.tile([128, KD, MT], BF16, tag="hn", bufs=1)
                    nc.sync.dma_start(hn_t[:], src_ap[:, :, ts(mi, MT)])
                    ssq = ps.tile([1, MT], F32, tag="ps")
                    for kb in range(KD // 2):
                        sq_t = sb.tile([128, 2, MT], BF16, tag="sq", bufs=2)
                        nc.scalar.square(sq_t[:], hn_t[:, 2 * kb:2 * kb + 2, :])
                        for j in range(2):
                            nc.tensor.matmul(
                                ssq[:], ones_sq[:], sq_t[:, j, :],
                                start=(kb == 0 and j == 0),
                                stop=(kb == KD // 2 - 1 and j == 1))
                    sqv = sb.tile([1, MT], F32, tag="sqv", bufs=2)
                    nc.scalar.activation(sqv[:], ssq[:], AF.Sqrt,
                                         bias=eps_t[:], scale=1.0 / DIM)
                    rstd = sb.tile([1, MT], F32, tag="rstd", bufs=2)
                    nc.vector.reciprocal(rstd[:], sqv[:])
                    bc = ps.tile([128, MT], F32, tag="ps")
                    nc.tensor.matmul(bc[:], ones_bc[:], rstd[:],
                                     start=True, stop=True)
                    for ko in range(KD):
                        nc.vector.tensor_tensor(
                            out=xn[:, ko, ts(mi, MT)], in0=hn_t[:, ko, :],
                            in1=bc[:], op=OP.mult)

            def rope_half(psrc, dst_a, dst_b, coff, mi):
                """psrc: [64, MT] psum (a rows 0:32, b rows 32:64).
                dst_a/dst_b: [32, MT] bf16 out. coff: 0 for q tables, 64 for k."""
                a = psrc[0:32, :]
                b = psrc[32:64, :]
                c_ = cosst[coff:coff + 32, ts(mi, MT)]
                s_ = cosst[coff + 32:coff + 64, ts(mi, MT)]
                t1 = sb.tile([32, MT], BF16, tag="rt1", bufs=2)
                t2 = sb.tile([32, MT], BF16, tag="rt2", bufs=2)
                nc.vector.tensor_tensor(out=t1[:], in0=a, in1=c_, op=OP.mult)
                nc.vector.tensor_tensor(out=t2[:], in0=b, in1=s_, op=OP.mult)
                nc.vector.tensor_tensor(out=dst_a, in0=t1[:], in1=t2[:],
                                        op=OP.subtract)
                t3 = sb.tile([32, MT], BF16, tag="rt1", bufs=2)
                t4 = sb.tile([32, MT], BF16, tag="rt2", bufs=2)
                nc.vector.tensor_tensor(out=t3[:], in0=a, in1=s_, op=OP.mult)
                nc.vector.tensor_tensor(out=t4[:], in0=b, in1=c_, op=OP.mult)
                nc.vector.tensor_tensor(out=dst_b, in0=t3[:], in1=t4[:],
                                        op=OP.add)

            for l in range(L):
                # ============ attention ============
                if l == 0:
                    norm_pass(hn0_d[:])
                # (for l>0 norm_pass over previous ch2_out was done at loop end)

                q_sb = sb.tile([128, 2, M], BF16, tag="q", bufs=1)
                kk_sb = sb.tile([128, M], BF16, tag="k", bufs=1)
                v_sb = sb.tile([64, M], BF16, tag="v", bufs=1)

                # Q projection (2 nj blocks of 128 cols = 2 heads each)
                for nj in range(2):
                    wq_t = sb.tile([128, KD, 128], BF16, tag="wn", bufs=3)
                    nc.sync.dma_start(wq_t[:], wq_d[l, :, nj])
                    qps = [ps.tile([128, MT], F32, tag="ps", name=f"qps{l}_{nj}_{i}") for i in range(NMT)]
                    for ko in range(KD):
                        for mi in range(NMT):
                            nc.tensor.matmul(
                                qps[mi][:], wq_t[:, ko, :], xn[:, ko, ts(mi, MT)],
                                start=(ko == 0), stop=(ko == KD - 1))
                    for mi in range(NMT):
                        for hh in range(2):
                            base = hh * 64
                            rope_half(
                                qps[mi][base:base + 64, :],
                                q_sb[base:base + 32, nj, ts(mi, MT)],
                                q_sb[base + 32:base + 64, nj, ts(mi, MT)],
                                0, mi)
                # KV projection (wk|wv packed as 128 cols; separate 64-wide GEMMs
                # so both k and v psums land at base partition 0)
                wkv_t = sb.tile([128, KD, 128], BF16, tag="wn", bufs=3)
                nc.sync.dma_start(wkv_t[:], wkv_d[l])
                kps = [ps.tile([64, MT], F32, tag="ps", name=f"kps{l}_{i}") for i in range(NMT)]
                vps = [ps.tile([64, MT], F32, tag="ps", name=f"vps{l}_{i}") for i in range(NMT)]
                for ko in range(KD):
                    for mi in range(NMT):
                        nc.tensor.matmul(
                            kps[mi][:], wkv_t[:, ko, 0:64], xn[:, ko, ts(mi, MT)],
                            start=(ko == 0), stop=(ko == KD - 1))
                    for mi in range(NMT):
                        nc.tensor.matmul(
                            vps[mi][:], wkv_t[:, ko, 64:128], xn[:, ko, ts(mi, MT)],
                            start=(ko == 0), stop=(ko == KD - 1))
                for mi in range(NMT):
                    rope_half(kps[mi][:],
                              kk_sb[0:32, ts(mi, MT)], kk_sb[32:64, ts(mi, MT)],
                              64, mi)
                    nc.vector.tensor_copy(v_sb[:, ts(mi, MT)], vps[mi][:])
                # duplicate k rows at base 64 so scores matmul can pair with
                # q head slices living at partitions 64-127
                nc.sync.dma_start(kk_sb[64:128, :], kk_sb[0:64, :])

                # transpose V into token-major chunks [t128, 64]
                vt_sb = sb.tile([128, B * 8, HD], BF16, tag="vt", bufs=1)
                for bt in range(B * 8):
                    vp = ps.tile([128, HD], BF16, tag="ps", name=f"vp{l}_{bt}")
                    nc.tensor.transpose(vp[:], v_sb[:, bt * 128:(bt + 1) * 128],
                                        ident[0:64, 0:64])
                    nc.vector.tensor_copy(vt_sb[:, bt, :], vp[:])

                o_sb = sb.tile([128, 2, M], BF16, tag="o", bufs=1)
                for b in range(B):
                    for si in range(8):
                        t_len = (si + 1) * 128
                        nsc = (t_len + 511) // 512
                        for qh in range(QH):
                            qb = (qh % 2) * 64
                            lq = q_sb[qb:qb + 64, qh // 2,
                                      b * S + si * 128: b * S + (si + 1) * 128]
                            sc = []
                            for tj in range(nsc):
                                tw = min(512, t_len - tj * 512)
                                sct = ps.tile([128, 512], F32, tag="ps",
                                              name=f"sc{l}_{b}_{si}_{qh}_{tj}")
                                nc.tensor.matmul(
                                    sct[:, 0:tw], lq,
                                    kk_sb[qb:qb + 64,
                                          b * S + tj * 512: b * S + tj * 512 + tw],
                                    start=True, stop=True)
                                sc.append((sct, tw))
                            tjd, off = divmod(si * 128, 512)
                            nc.vector.tensor_tensor(
                                out=sc[tjd][0][:, off:off + 128],
                                in0=sc[tjd][0][:, off:off + 128],
                                in1=maskA[:], op=OP.add)
                            nm = sb.tile([128, 2], F32, tag="nm", bufs=2)
                            for tj, (sct, tw) in enumerate(sc):
                                nc.vector.reduce_max(
                                    nm[:, tj:tj + 1], sct[:, 0:tw],
                                    axis=mybir.AxisListType.X, negate=True)
                            nmf = sb.tile([128, 1], F32, tag="nmf", bufs=2)
                            nc.vector.tensor_reduce(
                                nmf[:], nm[:, 0:nsc],
                                axis=mybir.AxisListType.X, op=OP.min)
                            den = sb.tile([128, 2], F32, tag="den", bufs=2)
                            p_t = sb.tile([128, 1024], BF16, tag="p", bufs=2)
                            for tj, (sct, tw) in enumerate(sc):
                                nc.scalar.activation(
                                    p_t[:, tj * 512:tj * 512 + tw],
                                    sct[:, 0:tw], AF.Exp, bias=nmf[:],
                                    accum_out=den[:, tj:tj + 1])
                            dsum = sb.tile([128, 1], F32, tag="dsum", bufs=2)
                            nc.vector.reduce_sum(dsum[:], den[:, 0:nsc],
                                                 axis=mybir.AxisListType.X)
                            rden = sb.tile([128, 1], F32, tag="rden", bufs=2)
                            nc.vector.reciprocal(rden[:], dsum[:])
                            nc.vector.tensor_scalar_mul(
                                p_t[:, 0:t_len], p_t[:, 0:t_len], rden[:])
                            ptile = sb.tile([128, 8, 128], BF16, tag="pt", bufs=2,
                                            name=f"ptile{l}_{b}_{si}_{qh}")
                            for tc in range(si + 1):
                                ptp = ps.tile([128, 128], BF16, tag="ps",
                                              name=f"ptp{l}_{b}_{si}_{qh}_{tc}")
                                nc.tensor.transpose(
                                    ptp[:], p_t[:, tc * 128:(tc + 1) * 128],
                                    ident[:])
                                nc.vector.tensor_copy(ptile[:, tc, :], ptp[:])
                            ov = ps.tile([64, 128], F32, tag="ps",
                                         name=f"ov{l}_{b}_{si}_{qh}")
                            for tc in range(si + 1):
                                nc.tensor.matmul(
                                    ov[:], vt_sb[:, b * 8 + tc, :],
                                    ptile[:, tc, :],
                                    start=(tc == 0), stop=(tc == si))
                            nc.vector.tensor_copy(
                                o_sb[(qh % 2) * 64:(qh % 2) * 64 + 64, qh // 2,
                                     b * S + si * 128: b * S + (si + 1) * 128],
                                ov[:])

                # AllGather o
                co_in = dd.tile([256, M], BF16, tag="co_in", bufs=2)
                nc.sync.dma_start(
                    co_in.rearrange("(ko ki) m -> ki ko m", ki=128)[:], o_sb[:])
                co_out = dd.tile([DIM, M], BF16, tag="co_out", bufs=2,
                                 addr_space="Shared")
                nc.gpsimd.collective_compute(
                    "AllGather", OP.bypass, replica_groups=RG,
                    ins=[co_in[:].opt()], outs=[co_out[:].opt()])

                # wo GEMM (ko-outer, full-o contraction) + residual
                wops = [ps.tile([128, MT], F32, tag="ps", name=f"wops{l}_{i}") for i in range(8)]
                co_r = co_out.rearrange("(ko ki) m -> ki ko m", ki=128)
                for kb in range(KD // 4):
                    wo_t = sb.tile([128, 4, 256], BF16, tag="wk2", bufs=2)
                    nc.sync.dma_start(wo_t[:], wo_d[l, :, kb * 4:(kb + 1) * 4, :])
                    for j in range(4):
                        ko = kb * 4 + j
                        ot = sb.tile([128, M], BF16, tag="kst", bufs=2)
                        nc.sync.dma_start(ot[:], co_r[:, ko, :])
                        for njj in range(2):
                            for mi in range(NMT):
                                nc.tensor.matmul(
                                    wops[mi * 2 + njj][:],
                                    wo_t[:, j, njj * 128:(njj + 1) * 128],
                                    ot[:, ts(mi, MT)],
                                    start=(ko == 0), stop=(ko == KD - 1))
                h1 = sb.tile([128, 2, M], F32, tag="h1", bufs=1)
                ch1_in = dd.tile([256, M], BF16, tag="ch1_in", bufs=2)
                ch1_r = ch1_in.rearrange("(ko ki) m -> ki ko m", ki=128)
                for mi in range(NMT):
                    for njj in range(2):
                        nc.vector.tensor_tensor(
                            out=h1[:, njj, ts(mi, MT)],
                            in0=wops[mi * 2 + njj][:],
                            in1=cur_h[:, njj, ts(mi, MT)], op=OP.add)
                        hb = sb.tile([128, MT], BF16, tag="hb", bufs=3)
                        nc.vector.tensor_copy(hb[:], h1[:, njj, ts(mi, MT)])
                        nc.sync.dma_start(ch1_r[:, njj, ts(mi, MT)], hb[:])
                ch1_out = dd.tile([DIM, M], BF16, tag="ch1_out", bufs=2,
                                  addr_space="Shared")
                nc.gpsimd.collective_compute(
                    "AllGather", OP.bypass, replica_groups=RG,
                    ins=[ch1_in[:].opt()], outs=[ch1_out[:].opt()])

                # ============ FFN ============
                norm_pass(ch1_out.rearrange("(ko ki) m -> ki ko m", ki=128))

                ca_in = dd.tile([FFN_PAD, M], BF16, tag="ca_in", bufs=2)
                ca_inr = ca_in.rearrange("(nj ki) m -> ki nj m", ki=128)
                for nj in range(6):
                    w1_t = sb.tile([128, KD, 128], BF16, tag="wn", bufs=3)
                    nc.sync.dma_start(w1_t[:], w1_d[l, :, nj])
                    w3_t = sb.tile([128, KD, 128], BF16, tag="wn", bufs=3)
                    nc.sync.dma_start(w3_t[:], w3_d[l, :, nj])
                    aps = [ps.tile([128, MT], F32, tag="ps", name=f"aps{l}_{nj}_{i}") for i in range(NMT)]
                    bps = [ps.tile([128, MT], F32, tag="ps", name=f"bps{l}_{nj}_{i}") for i in range(NMT)]
                    for ko in range(KD):
                        for mi in range(NMT):
                            nc.tensor.matmul(
                                aps[mi][:], w1_t[:, ko, :], xn[:, ko, ts(mi, MT)],
                                start=(ko == 0), stop=(ko == KD - 1))
                        for mi in range(NMT):
                            nc.tensor.matmul(
                                bps[mi][:], w3_t[:, ko, :], xn[:, ko, ts(mi, MT)],
                                start=(ko == 0), stop=(ko == KD - 1))
                    for mi in range(NMT):
                        sil = sb.tile([128, MT], BF16, tag="sil", bufs=2)
                        nc.scalar.activation(sil[:], aps[mi][:], AF.Silu)
                        at = sb.tile([128, MT], BF16, tag="at", bufs=3)
                        nc.vector.tensor_tensor(out=at[:], in0=bps[mi][:],
                                                in1=sil[:], op=OP.mult)
                        nc.sync.dma_start(ca_inr[:, nj, ts(mi, MT)], at[:])
                ca_out = dd.tile([HID_PAD, M], BF16, tag="ca_out", bufs=2,
                                 addr_space="Shared")
                nc.gpsimd.collective_compute(
                    "AllGather", OP.bypass, replica_groups=RG,
                    ins=[ca_in[:].opt()], outs=[ca_out[:].opt()])

                # w2 GEMM (ko-outer over padded hidden 6144) + residual
                w2ps = [ps.tile([128, MT], F32, tag="ps", name=f"w2ps{l}_{i}") for i in range(8)]
                ca_r = ca_out.rearrange("(ko ki) m -> ki ko m", ki=128)
                for kb in range(W2KD // 4):
                    w2_t = sb.tile([128, 4, 256], BF16, tag="wk2", bufs=2)
                    nc.sync.dma_start(w2_t[:], w2_d[l, :, kb * 4:(kb + 1) * 4, :])
                    for j in range(4):
                        ko = kb * 4 + j
                        at2 = sb.tile([128, M], BF16, tag="kst", bufs=2)
                        nc.sync.dma_start(at2[:], ca_r[:, ko, :])
                        for njj in range(2):
                            for mi in range(NMT):
                                nc.tensor.matmul(
                                    w2ps[mi * 2 + njj][:],
                                    w2_t[:, j, njj * 128:(njj + 1) * 128],
                                    at2[:, ts(mi, MT)],
                                    start=(ko == 0), stop=(ko == W2KD - 1))
                new_h = sb.tile([128, 2, M], F32, tag="h2", bufs=1)
                ch2_in = dd.tile([256, M], BF16, tag="ch2_in", bufs=2)
                ch2_r = ch2_in.rearrange("(ko ki) m -> ki ko m", ki=128)
                for mi in range(NMT):
                    for njj in range(2):
                        nc.vector.tensor_tensor(
                            out=new_h[:, njj, ts(mi, MT)],
                            in0=w2ps[mi * 2 + njj][:],
                            in1=h1[:, njj, ts(mi, MT)], op=OP.add)
                        hb = sb.tile([128, MT], BF16, tag="hb", bufs=3)
                        nc.vector.tensor_copy(hb[:], new_h[:, njj, ts(mi, MT)])
                        nc.sync.dma_start(ch2_r[:, njj, ts(mi, MT)], hb[:])
                cur_h = new_h
                ch2_out = dd.tile([DIM, M], BF16, tag="ch2_out", bufs=2,
                                  addr_space="Shared")
                nc.gpsimd.collective_compute(
                    "AllGather", OP.bypass, replica_groups=RG,
                    ins=[ch2_in[:].opt()], outs=[ch2_out[:].opt()])

                norm_pass(ch2_out.rearrange("(ko ki) m -> ki ko m", ki=128))

            # ============ output head ============
            for nf in range(NF):
                wt = sb.tile([128, KD, 128], BF16, tag="wn", bufs=3)
                nc.sync.dma_start(wt[:], wout_d[:, nf])
                hps = [ps.tile([128, MT], F32, tag="ps", name=f"hps{nf}_{i}") for i in range(NMT)]
                for ko in range(KD):
                    for mi in range(NMT):
                        nc.tensor.matmul(
                            hps[mi][:], wt[:, ko, :], xn[:, ko, ts(mi, MT)],
                            start=(ko == 0), stop=(ko == KD - 1))
                for mi in range(NMT):
                    lg = sb.tile([128, MT], F32, tag="lg", bufs=2,
                                 name=f"lg{nf}_{mi}")
                    nc.scalar.copy(lg[:], hps[mi][:])
                    nc.sync.dma_start(
                        logT_d[nf * 128:(nf + 1) * 128, ts(mi, MT)], lg[:])

    nc.compile()
    return nc


_ROPE_PERM = np.concatenate([np.arange(0, HD, 2), np.arange(1, HD, 2)])


def _perm_heads(w):
    """Permute rope pairs within each 64-col head block. w: [K, n_heads*64]."""
    K, N = w.shape
    return np.ascontiguousarray(
        w.reshape(K, N // HD, HD)[:, :, _ROPE_PERM].reshape(K, N))


def _pack_k(w):
    """[K, N] -> [128, K//128, N] with feature f = ko*128 + ki."""
    K, N = w.shape
    return np.ascontiguousarray(w.reshape(K // 128, 128, N).transpose(1, 0, 2))


def _pack_n(w, nblk=128):
    """[K, N] -> [128, N//nblk, K//128, nblk]."""
    K, N = w.shape
    x = w.reshape(K // 128, 128, N // nblk, nblk)
    return np.ascontiguousarray(x.transpose(1, 2, 0, 3))


def _prep_inputs(inputs):
    f32 = np.float32
    tokens = np.asarray(inputs["tokens"]).astype(np.int64).reshape(-1)
    emb = np.asarray(inputs["emb_W"], dtype=f32)
    wq = np.asarray(inputs["wq"], dtype=f32)
    wk = np.asarray(inputs["wk"], dtype=f32)
    wv = np.asarray(inputs["wv"], dtype=f32)
    wo = np.asarray(inputs["wo"], dtype=f32)
    w1 = np.asarray(inputs["w1"], dtype=f32)
    w2 = np.asarray(inputs["w2"], dtype=f32)
    w3 = np.asarray(inputs["w3"], dtype=f32)
    an = np.asarray(inputs["attn_norm_w"], dtype=f32)
    fn = np.asarray(inputs["ffn_norm_w"], dtype=f32)
    nw = np.asarray(inputs["norm_w"], dtype=f32)
    outw = np.asarray(inputs["out_W"], dtype=f32)
    cos = np.asarray(inputs["freqs_cos"], dtype=f32)
    sin = np.asarray(inputs["freqs_sin"], dtype=f32)

    h0T = np.ascontiguousarray(emb[tokens].T)          # [2048, 2048] f32
    hn0 = _pack_k(h0T).astype(nbf16)                   # [128, 16, 2048]

    ct = np.concatenate([cos.T, cos.T], axis=1)        # [32, 2048]
    st = np.concatenate([sin.T, sin.T], axis=1)
    scale = 1.0 / np.sqrt(HD).astype(f32)
    cosst = np.concatenate([scale * ct, scale * st, ct, st], axis=0).astype(nbf16)

    tri = np.tril(np.ones((128, 128), dtype=bool))
    maskA = np.where(tri, 0.0, -1e30).astype(f32)

    wq_f = wq * an[:, :, None]
    wk_f = wk * an[:, :, None]
    wv_f = wv * an[:, :, None]
    w1_f = w1 * fn[:, :, None]
    w3_f = w3 * fn[:, :, None]
    outw_f = outw * nw[:, None]

    in_maps = []
    for r in range(NCORES):
        m = {
            "hn0": hn0,
            "h0r": np.ascontiguousarray(
                h0T[r * 256:(r + 1) * 256].reshape(2, 128, M).transpose(1, 0, 2)),
            "cosst": cosst,
            "maskA": maskA,
        }
        wq_l, wkv_l, wo_l, w1_l, w3_l, w2_l = [], [], [], [], [], []
        for l in range(L):
            wq_r = _perm_heads(wq_f[l][:, r * 256:(r + 1) * 256])
            wq_l.append(_pack_n(wq_r.astype(nbf16)))
            wk_r = _perm_heads(wk_f[l][:, r * 64:(r + 1) * 64])
            wv_r = wv_f[l][:, r * 64:(r + 1) * 64]
            wkv_l.append(_pack_n(
                np.concatenate([wk_r, wv_r], axis=1).astype(nbf16))[:, 0])
            wo_l.append(_pack_k(
                wo[l][:, r * 256:(r + 1) * 256].astype(nbf16)))
            w1_r = np.zeros((DIM, FFN_PAD), dtype=f32)
            w1_r[:, :FFN_N] = w1_f[l][:, r * FFN_N:(r + 1) * FFN_N]
            w1_l.append(_pack_n(w1_r.astype(nbf16)))
            w3_r = np.zeros((DIM, FFN_PAD), dtype=f32)
            w3_r[:, :FFN_N] = w3_f[l][:, r * FFN_N:(r + 1) * FFN_N]
            w3_l.append(_pack_n(w3_r.astype(nbf16)))
            w2_r = np.zeros((HID_PAD, 256), dtype=f32)
            for jr in range(NCORES):
                w2_r[jr * FFN_PAD:jr * FFN_PAD + FFN_N] = \
                    w2[l][jr * FFN_N:(jr + 1) * FFN_N, r * 256:(r + 1) * 256]
            w2_l.append(_pack_k(w2_r.astype(nbf16)))
        m["wq"] = np.stack(wq_l)
        m["wkv"] = np.stack(wkv_l)
        m["wo"] = np.stack(wo_l)
        m["w1"] = np.stack(w1_l)
        m["w3"] = np.stack(w3_l)
        m["w2"] = np.stack(w2_l)
        wout_r = np.zeros((DIM, OUT_PAD), dtype=f32)
        wout_r[:, :OUT_N] = outw_f[:, r * OUT_N:(r + 1) * OUT_N]
        m["wout"] = _pack_n(wout_r.astype(nbf16))
        in_maps.append(m)
    return in_maps


_NC_CACHE = {}


def _get_nc():
    if "nc" not in _NC_CACHE:
        _NC_CACHE["nc"] = _build()
    return _NC_CACHE["nc"]


def run(inputs, trace=False):
    nc = _get_nc()
    in_maps = _prep_inputs(inputs)
    res = run_bass_kernel_spmd(nc, in_maps, core_ids=list(range(NCORES)),
                               trace=trace)
    logits = np.empty((M, VOCAB_TOTAL), dtype=np.float32)
    for r in range(NCORES):
        lt = res.results[r]["logT"]
        logits[:, r * OUT_N:(r + 1) * OUT_N] = lt[:OUT_N].T
    return logits.reshape(B, S, VOCAB_TOTAL), res


VOCAB_TOTAL = 32000


def kernel(**inputs):
    out, _ = run(inputs, trace=False)
    return out


# revision 13
# speedup vs baseline: 1.1973x; 1.1973x over previous
"""Llama forward-pass Trainium2 kernel: 8-core tensor-parallel (column-sharded
weights + AllGather between GEMM groups), bf16 matmuls with fp32 accumulation.

The two batch rows are independent streams: every phase and collective is
split per batch and emitted phase-skewed so one batch's PE work hides the
other's AllGather / softmax / norm latency.

Self-contained: hardcodes all shapes. kernel(**inputs) -> logits [2,1024,32000].
"""
import numpy as np
import ml_dtypes

import concourse.bass as bass
import concourse.bacc as bacc
import concourse.mybir as mybir
import concourse.tile as tile
from concourse.bass import ts
from concourse.bass_utils import run_bass_kernel_spmd
from concourse.masks import make_identity

AF = mybir.ActivationFunctionType
OP = mybir.AluOpType
BF16 = mybir.dt.bfloat16
F32 = mybir.dt.float32
nbf16 = ml_dtypes.bfloat16

NCORES = 8
L = 4
DIM = 2048
KD = DIM // 128            # 16
HD = 64
QH = 4                     # q heads per core
B, S = 2, 1024
M = B * S                  # 2048 tokens
MT = 512
NBT = 2                    # m tiles of 512 per batch
HIDDEN = 5632
FFN_N = 704                # per-core ffn cols
FFN_PAD = 768              # padded to 6 x 128
HID_PAD = FFN_PAD * NCORES # 6144
W2KD = HID_PAD // 128      # 48
OUT_N = 4000
OUT_PAD = 4096
NF = OUT_PAD // 128        # 32
EPS = 1e-5
RG = [list(range(NCORES))]
VOCAB_TOTAL = 32000


def _build():
    nc = bacc.Bacc("TRN2", target_bir_lowering=False, debug=False,
                   num_devices=NCORES)

    hn0_d = nc.dram_tensor("hn0", [128, KD, M], BF16, kind="ExternalInput")
    h0r_d = nc.dram_tensor("h0r", [128, 2, M], F32, kind="ExternalInput")
    rt_d = nc.dram_tensor("ropetab", [3, 128, S], BF16, kind="ExternalInput")
    mask_d = nc.dram_tensor("maskA", [128, 128], F32, kind="ExternalInput")
    wq_d = nc.dram_tensor("wq", [L, 128, 2, KD, 128], BF16, kind="ExternalInput")
    wkv_d = nc.dram_tensor("wkv", [L, 128, KD, 128], BF16, kind="ExternalInput")
    wo_d = nc.dram_tensor("wo", [L, 128, KD, 256], BF16, kind="ExternalInput")
    w1_d = nc.dram_tensor("w1", [L, 128, 6, KD, 128], BF16, kind="ExternalInput")
    w3_d = nc.dram_tensor("w3", [L, 128, 6, KD, 128], BF16, kind="ExternalInput")
    w2_d = nc.dram_tensor("w2", [L, 128, W2KD, 256], BF16, kind="ExternalInput")
    wout_d = nc.dram_tensor("wout", [128, NF, KD, 128], BF16, kind="ExternalInput")
    logT_d = nc.dram_tensor("logT", [OUT_PAD, M], F32, kind="ExternalOutput")

    with tile.TileContext(nc) as tc:
        with (
            tc.tile_pool(name="cst", bufs=1) as cst,
            tc.tile_pool(name="sb", bufs=2) as sb,
            tc.tile_pool(name="ps", bufs=1, space="PSUM") as ps,
            tc.tile_pool(name="dd", bufs=2, space="DRAM") as dd,
        ):
            # ---- constants ----
            TA = cst.tile([128, S], BF16)
            nc.sync.dma_start(TA[:], rt_d[0])
            TB = cst.tile([128, S], BF16)
            nc.sync.dma_start(TB[:], rt_d[1])
            TK = cst.tile([128, S], BF16)
            nc.sync.dma_start(TK[:], rt_d[2])
            maskA = cst.tile([128, 128], F32)
            nc.sync.dma_start(maskA[:], mask_d[:])
            ident = cst.tile([128, 128], BF16)
            make_identity(nc, ident[:])
            ones_sq = cst.tile([128, 1], BF16)
            nc.vector.memset(ones_sq[:], 1.0)
            ones_bc = cst.tile([1, 128], F32)
            nc.vector.memset(ones_bc[:], 1.0)
            eps_t = cst.tile([1, 1], F32)
            nc.vector.memset(eps_t[:], EPS)

            # ---- persistent per-batch activations ----
            xn = [sb.tile([128, KD, S], BF16, tag=f"xn{b}", bufs=1,
                          name=f"xn_{b}") for b in range(B)]
            cur_h = [sb.tile([128, 2, S], F32, tag=f"h2_{b}", bufs=1,
                             name=f"h0_{b}") for b in range(B)]
            for b in range(B):
                nc.sync.dma_start(cur_h[b][:], h0r_d[:, :, ts(b, S)])

            def norm_pass(b, src_ap, pfx):
                """src_ap: [128, KD, S] bf16 view for this batch.
                Writes xn[b] = src * rsqrt(mean sq + eps)."""
                for mi in range(NBT):
                    hn_t = sb.tile([128, KD, MT], BF16, tag="hn", bufs=1,
                                   name=f"{pfx}hn{b}_{mi}")
                    nc.sync.dma_start(hn_t[:], src_ap[:, :, ts(mi, MT)])
                    ssq = ps.tile([1, MT], F32, tag="sm", bufs=2, name=f"{pfx}ssq{b}_{mi}")
                    for kb in range(KD // 2):
                        sq_t = sb.tile([128, 2, MT], BF16, tag="sq", bufs=2,
                                       name=f"{pfx}sq{b}_{mi}_{kb}")
                        nc.scalar.square(sq_t[:], hn_t[:, 2 * kb:2 * kb + 2, :])
                        for j in range(2):
                            nc.tensor.matmul(
                                ssq[:], ones_sq[:], sq_t[:, j, :],
                                start=(kb == 0 and j == 0),
                                stop=(kb == KD // 2 - 1 and j == 1))
                    sqv = sb.tile([1, MT], F32, tag="sqv", bufs=1,
                                  name=f"{pfx}sqv{b}_{mi}")
                    nc.scalar.activation(sqv[:], ssq[:], AF.Sqrt,
                                         bias=eps_t[:], scale=1.0 / DIM)
                    rstd = sb.tile([1, MT], F32, tag="rstd", bufs=1,
                                   name=f"{pfx}rstd{b}_{mi}")
                    nc.vector.reciprocal(rstd[:], sqv[:])
                    bc = ps.tile([128, MT], F32, tag="sm", bufs=2, name=f"{pfx}bc{b}_{mi}")
                    nc.tensor.matmul(bc[:], ones_bc[:], rstd[:],
                                     start=True, stop=True)
                    for ko in range(KD):
                        nc.vector.tensor_tensor(
                            out=xn[b][:, ko, ts(mi, MT)], in0=hn_t[:, ko, :],
                            in1=bc[:], op=OP.mult)

            def rope_q(psrc, b, nj, mi, q_t, pfx):
                """psrc: [128, MT] psum with 2 heads [a;b][a;b]. Writes q_t.
                uc (SBUF) = x*cos, us (PSUM) = x*sin; combines mix one SBUF +
                one PSUM input so partition bases are unconstrained."""
                uc = sb.tile([128, MT], BF16, tag="ru", bufs=2,
                             name=f"{pfx}uc{b}_{nj}_{mi}")
                us = ps.tile([128, MT], F32, tag="sm", bufs=2,
                             name=f"{pfx}us{b}_{nj}_{mi}")
                tsl = ts(mi, MT)
                nc.vector.tensor_tensor(out=uc[:], in0=psrc[:], in1=TA[:, tsl],
                                        op=OP.mult)
                nc.vector.tensor_tensor(out=us[:], in0=psrc[:], in1=TB[:, tsl],
                                        op=OP.mult)
                for hh in range(2):
                    base = hh * 64
                    nc.vector.tensor_tensor(
                        out=q_t[base:base + 32, nj, tsl],
                        in0=uc[base:base + 32, :], in1=us[base + 32:base + 64, :],
                        op=OP.subtract)
                    nc.vector.tensor_tensor(
                        out=q_t[base + 32:base + 64, nj, tsl],
                        in0=us[base:base + 32, :], in1=uc[base + 32:base + 64, :],
                        op=OP.add)

            def rope_k(psrc, b, mi, kk_t, pfx):
                uc = sb.tile([64, MT], BF16, tag="ruk", bufs=2,
                             name=f"{pfx}uck{b}_{mi}")
                us = ps.tile([64, MT], F32, tag="sm", bufs=2,
                             name=f"{pfx}usk{b}_{mi}")
                tsl = ts(mi, MT)
                nc.vector.tensor_tensor(out=uc[:], in0=psrc[:], in1=TK[0:64, tsl],
                                        op=OP.mult)
                nc.vector.tensor_tensor(out=us[:], in0=psrc[:], in1=TK[64:128, tsl],
                                        op=OP.mult)
                nc.vector.tensor_tensor(out=kk_t[0:32, tsl], in0=uc[0:32, :],
                                        in1=us[32:64, :], op=OP.subtract)
                nc.vector.tensor_tensor(out=kk_t[32:64, tsl], in0=us[0:32, :],
                                        in1=uc[32:64, :], op=OP.add)

            for l in range(L):
                # ---- A: norm1 per batch ----
                if l == 0:
                    for b in range(B):
                        norm_pass(b, hn0_d[:, :, ts(b, S)], f"A{l}")
                # (l>0: norm over ch2_out emitted at end of previous layer)

                q_sb = [sb.tile([128, 2, S], BF16, tag=f"q{b}", bufs=1,
                                name=f"q{l}_{b}") for b in range(B)]
                kk_sb = [sb.tile([128, S], BF16, tag=f"k{b}", bufs=1,
                                 name=f"kk{l}_{b}") for b in range(B)]
                v_sb = [sb.tile([64, S], BF16, tag=f"v{b}", bufs=1,
                                name=f"v{l}_{b}") for b in range(B)]

                # ---- B: QKV + rope, weights loaded once, both batches use ----
                wq_t = {}
                for nj in range(2):
                    wq_t[nj] = sb.tile([128, KD, 128], BF16, tag="wn", bufs=3,
                                       name=f"wq{l}_{nj}")
                    nc.sync.dma_start(wq_t[nj][:], wq_d[l, :, nj])
                wkv_t = sb.tile([128, KD, 128], BF16, tag="wn", bufs=3,
                                name=f"wkv{l}")
                nc.sync.dma_start(wkv_t[:], wkv_d[l])
                for b in range(B):
                    for nj in range(2):
                        qps = [ps.tile([128, MT], F32, tag="acc", bufs=4,
                                       name=f"qps{l}_{b}_{nj}_{i}")
                               for i in range(NBT)]
                        for ko in range(KD):
                            for mi in range(NBT):
                                nc.tensor.matmul(
                                    qps[mi][:], wq_t[nj][:, ko, :],
                                    xn[b][:, ko, ts(mi, MT)],
                                    start=(ko == 0), stop=(ko == KD - 1))
                        for mi in range(NBT):
                            rope_q(qps[mi][:], b, nj, mi, q_sb[b], f"B{l}")
                    kps = [ps.tile([64, MT], F32, tag="acc", bufs=4,
                                   name=f"kps{l}_{b}_{i}") for i in range(NBT)]
                    vps = [ps.tile([64, MT], F32, tag="acc", bufs=4,
                                   name=f"vps{l}_{b}_{i}") for i in range(NBT)]
                    for ko in range(KD):
                        for mi in range(NBT):
                            nc.tensor.matmul(
                                kps[mi][:], wkv_t[:, ko, 0:64],
                                xn[b][:, ko, ts(mi, MT)],
                                start=(ko == 0), stop=(ko == KD - 1))
                        for mi in range(NBT):
                            nc.tensor.matmul(
                                vps[mi][:], wkv_t[:, ko, 64:128],
                                xn[b][:, ko, ts(mi, MT)],
                                start=(ko == 0), stop=(ko == KD - 1))
                    for mi in range(NBT):
                        rope_k(kps[mi][:], b, mi, kk_sb[b], f"B{l}")
                        nc.vector.tensor_copy(v_sb[b][:, ts(mi, MT)], vps[mi][:])
                    nc.sync.dma_start(kk_sb[b][64:128, :], kk_sb[b][0:64, :])

                # ---- C: attention + D: AllGather o, per batch ----
                co_out = []
                for b in range(B):
                    vt_sb = sb.tile([128, 8, HD], BF16, tag=f"vt{b}", bufs=1,
                                    name=f"vt{l}_{b}")
                    for bt in range(8):
                        vp = ps.tile([128, HD], BF16, tag="sm", bufs=2,
                                     name=f"vp{l}_{b}_{bt}")
                        nc.tensor.transpose(vp[:],
                                            v_sb[b][:, bt * 128:(bt + 1) * 128],
                                            ident[0:64, 0:64])
                        nc.vector.tensor_copy(vt_sb[:, bt, :], vp[:])
                    o_sb = sb.tile([128, 2, S], BF16, tag=f"o{b}", bufs=1,
                                   name=f"o{l}_{b}")
                    for si in range(8):
                        t_len = (si + 1) * 128
                        nsc = (t_len + 511) // 512
                        for qh in range(QH):
                            qb = (qh % 2) * 64
                            lq = q_sb[b][qb:qb + 64, qh // 2,
                                         si * 128:(si + 1) * 128]
                            sc = ps.tile([128, 1024], F32, tag="sc", bufs=1,
                                         name=f"sc{l}_{b}_{si}_{qh}")
                            for tj in range(nsc):
                                tw = min(512, t_len - tj * 512)
                                nc.tensor.matmul(
                                    sc[:, tj * 512:tj * 512 + tw], lq,
                                    kk_sb[b][qb:qb + 64, tj * 512:tj * 512 + tw],
                                    start=True, stop=True)
                            nc.vector.tensor_tensor(
                                out=sc[:, si * 128:si * 128 + 128],
                                in0=sc[:, si * 128:si * 128 + 128],
                                in1=maskA[:], op=OP.add)
                            nmf = sb.tile([128, 1], F32, tag="nmf", bufs=2,
                                          name=f"nmf{l}_{b}_{si}_{qh}")
                            nc.vector.reduce_max(nmf[:], sc[:, 0:t_len],
                                                 axis=mybir.AxisListType.X,
                                                 negate=True)
                            den = sb.tile([128, 1], F32, tag="den", bufs=2,
                                          name=f"den{l}_{b}_{si}_{qh}")
                            p_t = sb.tile([128, 1024], BF16, tag=f"p{b}", bufs=2,
                                          name=f"p{l}_{b}_{si}_{qh}")
                            nc.scalar.activation(
                                p_t[:, 0:t_len], sc[:, 0:t_len], AF.Exp,
                                bias=nmf[:], accum_out=den[:])
                            rden = sb.tile([128, 1], F32, tag="rden", bufs=2,
                                           name=f"rden{l}_{b}_{si}_{qh}")
                            nc.vector.reciprocal(rden[:], den[:])
                            nc.vector.tensor_scalar_mul(
                                p_t[:, 0:t_len], p_t[:, 0:t_len], rden[:])
                            ptile = sb.tile([128, 8, 128], BF16, tag=f"pt{b}",
                                            bufs=2, name=f"ptile{l}_{b}_{si}_{qh}")
                            for tc in range(si + 1):
                                ptp = ps.tile([128, 128], BF16, tag="sm", bufs=2,
                                              name=f"ptp{l}_{b}_{si}_{qh}_{tc}")
                                nc.tensor.transpose(
                                    ptp[:], p_t[:, tc * 128:(tc + 1) * 128],
                                    ident[:])
                                nc.vector.tensor_copy(ptile[:, tc, :], ptp[:])
                            ov = ps.tile([64, 128], F32, tag="sm", bufs=2,
                                         name=f"ov{l}_{b}_{si}_{qh}")
                            for tc in range(si + 1):
                                nc.tensor.matmul(
                                    ov[:], vt_sb[:, tc, :], ptile[:, tc, :],
                                    start=(tc == 0), stop=(tc == si))
                            nc.vector.tensor_copy(
                                o_sb[qb:qb + 64, qh // 2,
                                     si * 128:(si + 1) * 128],
                                ov[:])
                    ci = dd.tile([256, S], BF16, tag=f"co_in{b}", bufs=2,
                                 name=f"co_in{l}_{b}")
                    nc.sync.dma_start(
                        ci.rearrange("(ko ki) m -> ki ko m", ki=128)[:], o_sb[:])
                    co = dd.tile([DIM, S], BF16, tag=f"co_out{b}", bufs=2,
                                 addr_space="Shared", name=f"co_out{l}_{b}")
                    nc.gpsimd.collective_compute(
                        "AllGather", OP.bypass, replica_groups=RG,
                        ins=[ci[:].opt()], outs=[co[:].opt()])
                    co_out.append(co)

                # ---- E: wo GEMM + residual, per batch (weights streamed)
                h1_sb, ch1_out = [], []
                for b in range(B):
                    wops = [ps.tile([128, MT], F32, tag="acc", bufs=4,
                                    name=f"wops{l}_{b}_{i}") for i in range(4)]
                    co_r = co_out[b].rearrange("(ko ki) m -> ki ko m", ki=128)
                    for kb in range(4):
                        wo_t = sb.tile([128, 4, 256], BF16, tag="wk2", bufs=2,
                                       name=f"wo{l}_{b}_{kb}")
                        nc.sync.dma_start(wo_t[:],
                                          wo_d[l, :, kb * 4:(kb + 1) * 4, :])
                        for j in range(4):
                            ko = kb * 4 + j
                            ot = sb.tile([128, S], BF16, tag="kst", bufs=2,
                                         name=f"ot{l}_{b}_{ko}")
                            nc.sync.dma_start(ot[:], co_r[:, ko, :])
                            for njj in range(2):
                                for mi in range(NBT):
                                    nc.tensor.matmul(
                                        wops[mi * 2 + njj][:],
                                        wo_t[:, j, njj * 128:(njj + 1) * 128],
                                        ot[:, ts(mi, MT)],
                                        start=(ko == 0), stop=(ko == KD - 1))
                    h1 = sb.tile([128, 2, S], F32, tag=f"h1_{b}", bufs=1,
                                 name=f"h1_{l}_{b}")
                    ci = dd.tile([256, S], BF16, tag=f"ch1_in{b}", bufs=2,
                                 name=f"ch1_in{l}_{b}")
                    ci_r = ci.rearrange("(ko ki) m -> ki ko m", ki=128)
                    for mi in range(NBT):
                        for njj in range(2):
                            nc.vector.tensor_tensor(
                                out=h1[:, njj, ts(mi, MT)],
                                in0=wops[mi * 2 + njj][:],
                                in1=cur_h[b][:, njj, ts(mi, MT)], op=OP.add)
                            hb = sb.tile([128, MT], BF16, tag="hb", bufs=2,
                                         name=f"hb1_{l}_{b}_{mi}_{njj}")
                            nc.vector.tensor_copy(hb[:], h1[:, njj, ts(mi, MT)])
                            nc.sync.dma_start(ci_r[:, njj, ts(mi, MT)], hb[:])
                    co = dd.tile([DIM, S], BF16, tag=f"ch1_out{b}", bufs=2,
                                 addr_space="Shared", name=f"ch1_out{l}_{b}")
                    nc.gpsimd.collective_compute(
                        "AllGather", OP.bypass, replica_groups=RG,
                        ins=[ci[:].opt()], outs=[co[:].opt()])
                    h1_sb.append(h1)
                    ch1_out.append(co)

                # ---- G: norm2 per batch ----
                for b in range(B):
                    norm_pass(b, ch1_out[b].rearrange("(ko ki) m -> ki ko m",
                                                      ki=128), f"G{l}")

                # ---- H: w1/w3 GEMM (batches interleaved per nj block) ----
                ca_in = [dd.tile([FFN_PAD, S], BF16, tag=f"ca_in{b}", bufs=2,
                                 name=f"ca_in{l}_{b}") for b in range(B)]
                for nj in range(6):
                    w1_t = sb.tile([128, KD, 128], BF16, tag="wn", bufs=3,
                                   name=f"w1_{l}_{nj}")
                    nc.sync.dma_start(w1_t[:], w1_d[l, :, nj])
                    w3_t = sb.tile([128, KD, 128], BF16, tag="wn", bufs=3,
                                   name=f"w3_{l}_{nj}")
                    nc.sync.dma_start(w3_t[:], w3_d[l, :, nj])
                    for b in range(B):
                        aps = [ps.tile([128, MT], F32, tag="acc", bufs=4,
                                       name=f"aps{l}_{b}_{nj}_{i}")
                               for i in range(NBT)]
                        bps = [ps.tile([128, MT], F32, tag="acc", bufs=4,
                                       name=f"bps{l}_{b}_{nj}_{i}")
                               for i in range(NBT)]
                        for ko in range(KD):
                            for mi in range(NBT):
                                nc.tensor.matmul(
                                    aps[mi][:], w1_t[:, ko, :],
                                    xn[b][:, ko, ts(mi, MT)],
                                    start=(ko == 0), stop=(ko == KD - 1))
                            for mi in range(NBT):
                                nc.tensor.matmul(
                                    bps[mi][:], w3_t[:, ko, :],
                                    xn[b][:, ko, ts(mi, MT)],
                                    start=(ko == 0), stop=(ko == KD - 1))
                        ca_r = ca_in[b].rearrange("(nj ki) m -> ki nj m", ki=128)
                        for mi in range(NBT):
                            sil = sb.tile([128, MT], BF16, tag="sil", bufs=2,
                                          name=f"sil{l}_{b}_{nj}_{mi}")
                            nc.scalar.activation(sil[:], aps[mi][:], AF.Silu)
                            at = sb.tile([128, MT], BF16, tag="at", bufs=2,
                                         name=f"at{l}_{b}_{nj}_{mi}")
                            nc.vector.tensor_tensor(out=at[:], in0=bps[mi][:],
                                                    in1=sil[:], op=OP.mult)
                            nc.sync.dma_start(ca_r[:, nj, ts(mi, MT)], at[:])
                # ---- I: AllGather a per batch ----
                ca_out = []
                for b in range(B):
                    co = dd.tile([HID_PAD, S], BF16, tag=f"ca_out{b}", bufs=2,
                                 addr_space="Shared", name=f"ca_out{l}_{b}")
                    nc.gpsimd.collective_compute(
                        "AllGather", OP.bypass, replica_groups=RG,
                        ins=[ca_in[b][:].opt()], outs=[co[:].opt()])
                    ca_out.append(co)

                # ---- J: w2 GEMM per batch + residual ----
                new_h = [sb.tile([128, 2, S], F32, tag=f"h2_{b}", bufs=1,
                                 name=f"h2_{l}_{b}") for b in range(B)]
                for b in range(B):
                    w2ps = [ps.tile([128, MT], F32, tag="acc", bufs=4,
                                    name=f"w2ps{l}_{b}_{i}") for i in range(4)]
                    ca_r = ca_out[b].rearrange("(ko ki) m -> ki ko m", ki=128)
                    for kb in range(W2KD // 4):
                        w2_t = sb.tile([128, 4, 256], BF16, tag="wk2", bufs=2,
                                       name=f"w2_{l}_{b}_{kb}")
                        nc.sync.dma_start(w2_t[:],
                                          w2_d[l, :, kb * 4:(kb + 1) * 4, :])
                        for j in range(4):
                            ko = kb * 4 + j
                            at2 = sb.tile([128, S], BF16, tag="kst", bufs=2,
                                          name=f"at2_{l}_{b}_{ko}")
                            nc.sync.dma_start(at2[:], ca_r[:, ko, :])
                            for njj in range(2):
                                for mi in range(NBT):
                                    nc.tensor.matmul(
                                        w2ps[mi * 2 + njj][:],
                                        w2_t[:, j, njj * 128:(njj + 1) * 128],
                                        at2[:, ts(mi, MT)],
                                        start=(ko == 0), stop=(ko == W2KD - 1))
                    ci = dd.tile([256, S], BF16, tag=f"ch2_in{b}", bufs=2,
                                 name=f"ch2_in{l}_{b}")
                    ci_r = ci.rearrange("(ko ki) m -> ki ko m", ki=128)
                    for mi in range(NBT):
                        for njj in range(2):
                            nc.vector.tensor_tensor(
                                out=new_h[b][:, njj, ts(mi, MT)],
                                in0=w2ps[mi * 2 + njj][:],
                                in1=h1_sb[b][:, njj, ts(mi, MT)], op=OP.add)
                            hb = sb.tile([128, MT], BF16, tag="hb", bufs=2,
                                         name=f"hb2_{l}_{b}_{mi}_{njj}")
                            nc.vector.tensor_copy(hb[:],
                                                  new_h[b][:, njj, ts(mi, MT)])
                            nc.sync.dma_start(ci_r[:, njj, ts(mi, MT)], hb[:])
                    co = dd.tile([DIM, S], BF16, tag=f"ch2_out{b}", bufs=2,
                                 addr_space="Shared", name=f"ch2_out{l}_{b}")
                    nc.gpsimd.collective_compute(
                        "AllGather", OP.bypass, replica_groups=RG,
                        ins=[ci[:].opt()], outs=[co[:].opt()])
                    # next layer's norm1 (or the final norm) for this batch
                    norm_pass(b, co.rearrange("(ko ki) m -> ki ko m", ki=128),
                              f"K{l}")
                cur_h = new_h

            # ============ output head ============
            for nf in range(NF):
                wt = sb.tile([128, KD, 128], BF16, tag="wn", bufs=3,
                             name=f"wout{nf}")
                nc.sync.dma_start(wt[:], wout_d[:, nf])
                hps = [ps.tile([128, MT], F32, tag="acc", bufs=4,
                               name=f"hps{nf}_{i}") for i in range(4)]
                for ko in range(KD):
                    for gmi in range(4):
                        nc.tensor.matmul(
                            hps[gmi][:], wt[:, ko, :],
                            xn[gmi // 2][:, ko, ts(gmi % 2, MT)],
                            start=(ko == 0), stop=(ko == KD - 1))
                for gmi in range(4):
                    lg = sb.tile([128, MT], F32, tag="lg", bufs=2,
                                 name=f"lg{nf}_{gmi}")
                    nc.scalar.copy(lg[:], hps[gmi][:])
                    nc.sync.dma_start(
                        logT_d[nf * 128:(nf + 1) * 128, ts(gmi, MT)], lg[:])

    nc.compile()
    return nc


_ROPE_PERM = np.concatenate([np.arange(0, HD, 2), np.arange(1, HD, 2)])


def _perm_heads(w):
    """Permute rope pairs within each 64-col head block. w: [K, n_heads*64]."""
    K, N = w.shape
    return np.ascontiguousarray(
        w.reshape(K, N // HD, HD)[:, :, _ROPE_PERM].reshape(K, N))


def _pack_k(w):
    """[K, N] -> [128, K//128, N] with feature f = ko*128 + ki."""
    K, N = w.shape
    return np.ascontiguousarray(w.reshape(K // 128, 128, N).transpose(1, 0, 2))


def _pack_n(w, nblk=128):
    """[K, N] -> [128, N//nblk, K//128, nblk]."""
    K, N = w.shape
    x = w.reshape(K // 128, 128, N // nblk, nblk)
    return np.ascontiguousarray(x.transpose(1, 2, 0, 3))


def _prep_inputs(inputs):
    f32 = np.float32
    tokens = np.asarray(inputs["tokens"]).astype(np.int64).reshape(-1)
    emb = np.asarray(inputs["emb_W"], dtype=f32)
    wq = np.asarray(inputs["wq"], dtype=f32)
    wk = np.asarray(inputs["wk"], dtype=f32)
    wv = np.asarray(inputs["wv"], dtype=f32)
    wo = np.asarray(inputs["wo"], dtype=f32)
    w1 = np.asarray(inputs["w1"], dtype=f32)
    w2 = np.asarray(inputs["w2"], dtype=f32)
    w3 = np.asarray(inputs["w3"], dtype=f32)
    an = np.asarray(inputs["attn_norm_w"], dtype=f32)
    fn = np.asarray(inputs["ffn_norm_w"], dtype=f32)
    nw = np.asarray(inputs["norm_w"], dtype=f32)
    outw = np.asarray(inputs["out_W"], dtype=f32)
    cos = np.asarray(inputs["freqs_cos"], dtype=f32)
    sin = np.asarray(inputs["freqs_sin"], dtype=f32)

    h0T = np.ascontiguousarray(emb[tokens].T)          # [2048, 2048] f32
    hn0 = _pack_k(h0T).astype(nbf16)                   # [128, 16, 2048]

    ct = np.ascontiguousarray(cos.T).astype(f32)       # [32, 1024]
    st = np.ascontiguousarray(sin.T).astype(f32)
    scale = np.float32(1.0 / np.sqrt(HD))
    cq, sq = scale * ct, scale * st
    TA = np.concatenate([cq, cq, cq, cq], axis=0).astype(nbf16)
    TBt = np.concatenate([sq, sq, sq, sq], axis=0).astype(nbf16)
    TKt = np.concatenate([ct, ct, st, st], axis=0).astype(nbf16)
    ropetab = np.stack([TA, TBt, TKt])                 # [3, 128, 1024]

    tri = np.tril(np.ones((128, 128), dtype=bool))
    maskA = np.where(tri, 0.0, -1e30).astype(f32)

    wq_f = wq * an[:, :, None]
    wk_f = wk * an[:, :, None]
    wv_f = wv * an[:, :, None]
    w1_f = w1 * fn[:, :, None]
    w3_f = w3 * fn[:, :, None]
    outw_f = outw * nw[:, None]

    in_maps = []
    for r in range(NCORES):
        m = {
            "hn0": hn0,
            "h0r": np.ascontiguousarray(
                h0T[r * 256:(r + 1) * 256].reshape(2, 128, M).transpose(1, 0, 2)),
            "ropetab": ropetab,
            "maskA": maskA,
        }
        wq_l, wkv_l, wo_l, w1_l, w3_l, w2_l = [], [], [], [], [], []
        for l in range(L):
            wq_r = _perm_heads(wq_f[l][:, r * 256:(r + 1) * 256])
            wq_l.append(_pack_n(wq_r.astype(nbf16)))
            wk_r = _perm_heads(wk_f[l][:, r * 64:(r + 1) * 64])
            wv_r = wv_f[l][:, r * 64:(r + 1) * 64]
            wkv_l.append(_pack_n(
                np.concatenate([wk_r, wv_r], axis=1).astype(nbf16))[:, 0])
            wo_l.append(_pack_k(
                wo[l][:, r * 256:(r + 1) * 256].astype(nbf16)))
            w1_r = np.zeros((DIM, FFN_PAD), dtype=f32)
            w1_r[:, :FFN_N] = w1_f[l][:, r * FFN_N:(r + 1) * FFN_N]
            w1_l.append(_pack_n(w1_r.astype(nbf16)))
            w3_r = np.zeros((DIM, FFN_PAD), dtype=f32)
            w3_r[:, :FFN_N] = w3_f[l][:, r * FFN_N:(r + 1) * FFN_N]
            w3_l.append(_pack_n(w3_r.astype(nbf16)))
            w2_r = np.zeros((HID_PAD, 256), dtype=f32)
            for jr in range(NCORES):
                w2_r[jr * FFN_PAD:jr * FFN_PAD + FFN_N] = \
                    w2[l][jr * FFN_N:(jr + 1) * FFN_N, r * 256:(r + 1) * 256]
            w2_l.append(_pack_k(w2_r.astype(nbf16)))
        m["wq"] = np.stack(wq_l)
        m["wkv"] = np.stack(wkv_l)
        m["wo"] = np.stack(wo_l)
        m["w1"] = np.stack(w1_l)
        m["w3"] = np.stack(w3_l)
        m["w2"] = np.stack(w2_l)
        wout_r = np.zeros((DIM, OUT_PAD), dtype=f32)
        wout_r[:, :OUT_N] = outw_f[:, r * OUT_N:(r + 1) * OUT_N]
        m["wout"] = _pack_n(wout_r.astype(nbf16))
        in_maps.append(m)
    return in_maps


_NC_CACHE = {}


def _get_nc():
    if "nc" not in _NC_CACHE:
        _NC_CACHE["nc"] = _build()
    return _NC_CACHE["nc"]


def run(inputs, trace=False):
    nc = _get_nc()
    in_maps = _prep_inputs(inputs)
    res = run_bass_kernel_spmd(nc, in_maps, core_ids=list(range(NCORES)),
                               trace=trace)
    logits = np.empty((M, VOCAB_TOTAL), dtype=np.float32)
    for r in range(NCORES):
        lt = res.results[r]["logT"]
        logits[:, r * OUT_N:(r + 1) * OUT_N] = lt[:OUT_N].T
    return logits.reshape(B, S, VOCAB_TOTAL), res


def kernel(**inputs):
    out, _ = run(inputs, trace=False)
    return out


# revision 18
# speedup vs baseline: 1.2636x; 1.0554x over previous
"""Llama forward-pass Trainium2 kernel: 8-core tensor-parallel (column-sharded
weights + AllGather between GEMM groups), bf16 matmuls with fp32 accumulation.

The two batch rows are independent streams: every phase and collective is
split per batch and emitted phase-skewed so one batch's PE work hides the
other's AllGather / softmax / norm latency.

Self-contained: hardcodes all shapes. kernel(**inputs) -> logits [2,1024,32000].
"""
import numpy as np
import ml_dtypes

import concourse.bass as bass
import concourse.bacc as bacc
import concourse.mybir as mybir
import concourse.tile as tile
from concourse.bass import ts
from concourse.bass_utils import run_bass_kernel_spmd
from concourse.masks import make_identity

AF = mybir.ActivationFunctionType
OP = mybir.AluOpType
BF16 = mybir.dt.bfloat16
F32 = mybir.dt.float32
nbf16 = ml_dtypes.bfloat16

NCORES = 8
L = 4
DIM = 2048
KD = DIM // 128            # 16
HD = 64
QH = 4                     # q heads per core
B, S = 2, 1024
M = B * S                  # 2048 tokens
MT = 512
NBT = 2                    # m tiles of 512 per batch
HIDDEN = 5632
FFN_N = 704                # per-core ffn cols
FFN_PAD = 768              # padded to 6 x 128
HID_PAD = FFN_PAD * NCORES # 6144
W2KD = HID_PAD // 128      # 48
OUT_N = 4000
OUT_PAD = 4096
NF = OUT_PAD // 128        # 32
EPS = 1e-5
RG = [list(range(NCORES))]
VOCAB_TOTAL = 32000


def _build():
    nc = bacc.Bacc("TRN2", target_bir_lowering=False, debug=False,
                   num_devices=NCORES)

    hn0_d = nc.dram_tensor("hn0", [128, KD, M], BF16, kind="ExternalInput")
    h0r_d = nc.dram_tensor("h0r", [128, 2, M], F32, kind="ExternalInput")
    rt_d = nc.dram_tensor("ropetab", [3, 128, S], BF16, kind="ExternalInput")
    mask_d = nc.dram_tensor("maskA", [128, 128], F32, kind="ExternalInput")
    ssq0_d = nc.dram_tensor("ssq0", [1, M], F32, kind="ExternalInput")
    wq_d = nc.dram_tensor("wq", [L, 128, 2, KD, 128], BF16, kind="ExternalInput")
    wkv_d = nc.dram_tensor("wkv", [L, 128, KD, 128], BF16, kind="ExternalInput")
    wo_d = nc.dram_tensor("wo", [L, 128, KD, 256], BF16, kind="ExternalInput")
    w1_d = nc.dram_tensor("w1", [L, 128, 6, KD, 128], BF16, kind="ExternalInput")
    w3_d = nc.dram_tensor("w3", [L, 128, 6, KD, 128], BF16, kind="ExternalInput")
    w2_d = nc.dram_tensor("w2", [L, 128, W2KD, 256], BF16, kind="ExternalInput")
    wout_d = nc.dram_tensor("wout", [128, NF, KD, 128], BF16, kind="ExternalInput")
    logT_d = nc.dram_tensor("logT", [OUT_PAD, M], F32, kind="ExternalOutput")

    with tile.TileContext(nc) as tc:
        with (
            tc.tile_pool(name="cst", bufs=1) as cst,
            tc.tile_pool(name="sb", bufs=2) as sb,
            tc.tile_pool(name="ps", bufs=1, space="PSUM") as ps,
            tc.tile_pool(name="dd", bufs=2, space="DRAM") as dd,
        ):
            # ---- constants ----
            TA = cst.tile([128, S], BF16)
            nc.sync.dma_start(TA[:], rt_d[0])
            TB = cst.tile([128, S], BF16)
            nc.sync.dma_start(TB[:], rt_d[1])
            TK = cst.tile([128, S], BF16)
            nc.sync.dma_start(TK[:], rt_d[2])
            maskA = cst.tile([128, 128], F32)
            nc.sync.dma_start(maskA[:], mask_d[:])
            ident = cst.tile([128, 128], BF16)
            make_identity(nc, ident[:])
            ones_sq = cst.tile([128, 1], BF16)
            nc.vector.memset(ones_sq[:], 1.0)
            ones_bc = cst.tile([1, 128], F32)
            nc.vector.memset(ones_bc[:], 1.0)
            eps_t = cst.tile([1, 1], F32)
            nc.vector.memset(eps_t[:], EPS)

            # ---- persistent per-batch activations ----
            xn = [sb.tile([128, KD, S], BF16, tag=f"xn{b}", bufs=1,
                          name=f"xn_{b}") for b in range(B)]
            cur_h = [sb.tile([128, 2, S], F32, tag=f"h2_{b}", bufs=1,
                             name=f"h0_{b}") for b in range(B)]
            for b in range(B):
                nc.sync.dma_start(cur_h[b][:], h0r_d[:, :, ts(b, S)])

            def _norm_tail(b, mi, hn_t, ssq_ap, is_psum, pfx):
                """ssq_ap: [1, MT] total sum-of-squares (psum or sbuf)."""
                sqv = sb.tile([1, MT], F32, tag="sqv", bufs=1,
                              name=f"{pfx}sqv{b}_{mi}")
                nc.scalar.activation(sqv[:], ssq_ap, AF.Sqrt,
                                     bias=eps_t[:], scale=1.0 / DIM)
                rstd = sb.tile([1, MT], F32, tag="rstd", bufs=1,
                               name=f"{pfx}rstd{b}_{mi}")
                nc.vector.reciprocal(rstd[:], sqv[:])
                bc = ps.tile([128, MT], F32, tag="sm", bufs=2,
                             name=f"{pfx}bc{b}_{mi}")
                nc.tensor.matmul(bc[:], ones_bc[:], rstd[:],
                                 start=True, stop=True)
                for ko in range(KD):
                    nc.vector.tensor_tensor(
                        out=xn[b][:, ko, ts(mi, MT)], in0=hn_t[:, ko, :],
                        in1=bc[:], op=OP.mult)

            def norm_l0(b, pfx):
                """Layer-0 norm: ssq precomputed on host."""
                for mi in range(NBT):
                    hn_t = sb.tile([128, KD, MT], BF16, tag="hn", bufs=1,
                                   name=f"{pfx}hn{b}_{mi}")
                    nc.sync.dma_start(hn_t[:],
                                      hn0_d[:, :, b * S + mi * MT:
                                            b * S + (mi + 1) * MT])
                    s0 = sb.tile([1, MT], F32, tag="s0", bufs=2,
                                 name=f"{pfx}s0_{b}_{mi}")
                    nc.sync.dma_start(s0[:],
                                      ssq0_d[:, b * S + mi * MT:
                                             b * S + (mi + 1) * MT])
                    _norm_tail(b, mi, hn_t, s0[:], False, pfx)

            def norm_from_ag(b, co, pfx):
                """co: [NCORES*257, S] AG output carrying h rows + ssq row."""
                co3 = co.rearrange("(r x) m -> r x m", x=257)
                ssqp = sb.tile([8, S], BF16, tag="ssqp", bufs=2,
                               name=f"{pfx}ssqp{b}")
                nc.sync.dma_start(ssqp[:], co3[:, 256, :])
                hview = co3[:, 0:256, :].rearrange(
                    "r (ko ki) m -> ki r ko m", ki=128)
                for mi in range(NBT):
                    hn_t = sb.tile([128, KD, MT], BF16, tag="hn", bufs=1,
                                   name=f"{pfx}hn{b}_{mi}")
                    hn4 = hn_t[:].rearrange("p (r ko) m -> p r ko m", ko=2)
                    for ko_l in range(2):
                        nc.sync.dma_start(hn4[:, :, ko_l, :],
                                          hview[:, :, ko_l, ts(mi, MT)])
                    ssq = ps.tile([1, MT], F32, tag="sm", bufs=2,
                                  name=f"{pfx}ssq{b}_{mi}")
                    nc.tensor.matmul(ssq[:], ones_sq[0:8, :],
                                     ssqp[:, ts(mi, MT)],
                                     start=True, stop=True)
                    _norm_tail(b, mi, hn_t, ssq[:], True, pfx)

            def rope_q(psrc, b, nj, mi, q_t, pfx):
                """psrc: [128, MT] psum with 2 heads [a;b][a;b]. Writes q_t.
                uc (SBUF) = x*cos, us (PSUM) = x*sin; combines mix one SBUF +
                one PSUM input so partition bases are unconstrained."""
                uc = sb.tile([128, MT], BF16, tag="ru", bufs=2,
                             name=f"{pfx}uc{b}_{nj}_{mi}")
                us = ps.tile([128, MT], F32, tag="sm", bufs=2,
                             name=f"{pfx}us{b}_{nj}_{mi}")
                tsl = ts(mi, MT)
                nc.vector.tensor_tensor(out=uc[:], in0=psrc[:], in1=TA[:, tsl],
                                        op=OP.mult)
                nc.vector.tensor_tensor(out=us[:], in0=psrc[:], in1=TB[:, tsl],
                                        op=OP.mult)
                for hh in range(2):
                    base = hh * 64
                    nc.vector.tensor_tensor(
                        out=q_t[base:base + 32, nj, tsl],
                        in0=uc[base:base + 32, :], in1=us[base + 32:base + 64, :],
                        op=OP.subtract)
                    nc.vector.tensor_tensor(
                        out=q_t[base + 32:base + 64, nj, tsl],
                        in0=us[base:base + 32, :], in1=uc[base + 32:base + 64, :],
                        op=OP.add)

            def rope_k(psrc, b, mi, kk_t, pfx):
                uc = sb.tile([64, MT], BF16, tag="ruk", bufs=2,
                             name=f"{pfx}uck{b}_{mi}")
                us = ps.tile([64, MT], F32, tag="sm", bufs=2,
                             name=f"{pfx}usk{b}_{mi}")
                tsl = ts(mi, MT)
                nc.vector.tensor_tensor(out=uc[:], in0=psrc[:], in1=TK[0:64, tsl],
                                        op=OP.mult)
                nc.vector.tensor_tensor(out=us[:], in0=psrc[:], in1=TK[64:128, tsl],
                                        op=OP.mult)
                nc.vector.tensor_tensor(out=kk_t[0:32, tsl], in0=uc[0:32, :],
                                        in1=us[32:64, :], op=OP.subtract)
                nc.vector.tensor_tensor(out=kk_t[32:64, tsl], in0=us[0:32, :],
                                        in1=uc[32:64, :], op=OP.add)

            def emit_resid(l, b, which, hdst, psums, hres, ci):
                """hdst = psums + hres per (mi,njj); write bf16 rows + local
                ssq partial (row 256) into the AG input tile ci [257, S]."""
                ci_r = ci[0:256, :].rearrange("(ko ki) m -> ki ko m", ki=128)
                for mi in range(NBT):
                    for njj in range(2):
                        nc.vector.tensor_tensor(
                            out=hdst[:, njj, ts(mi, MT)],
                            in0=psums[mi * 2 + njj][:],
                            in1=hres[:, njj, ts(mi, MT)], op=OP.add)
                        hb = sb.tile([128, MT], BF16, tag="hb", bufs=2,
                                     name=f"hb{which}_{l}_{b}_{mi}_{njj}")
                        nc.vector.tensor_copy(hb[:], hdst[:, njj, ts(mi, MT)])
                        nc.sync.dma_start(ci_r[:, njj, ts(mi, MT)], hb[:])
                    sq_t = sb.tile([128, 2, MT], BF16, tag="sq", bufs=2,
                                   name=f"sqr{which}_{l}_{b}_{mi}")
                    nc.scalar.square(sq_t[:], hdst[:, :, ts(mi, MT)])
                    sps = ps.tile([1, MT], F32, tag="sm", bufs=2,
                                  name=f"sps{which}_{l}_{b}_{mi}")
                    for j in range(2):
                        nc.tensor.matmul(sps[:], ones_sq[:], sq_t[:, j, :],
                                         start=(j == 0), stop=(j == 1))
                    sqb = sb.tile([1, MT], BF16, tag="sqb", bufs=2,
                                  name=f"sqb{which}_{l}_{b}_{mi}")
                    nc.vector.tensor_copy(sqb[:], sps[:])
                    nc.sync.dma_start(ci[256:257, ts(mi, MT)], sqb[:])

            for l in range(L):
                # ---- A: norm1 per batch ----
                if l == 0:
                    for b in range(B):
                        norm_l0(b, f"A{l}")
                # (l>0: norm over ch2_out emitted at end of previous layer)

                q_sb = [sb.tile([128, 2, S], BF16, tag=f"q{b}", bufs=1,
                                name=f"q{l}_{b}") for b in range(B)]
                kk_sb = [sb.tile([128, S], BF16, tag=f"k{b}", bufs=1,
                                 name=f"kk{l}_{b}") for b in range(B)]
                v_sb = [sb.tile([64, S], BF16, tag=f"v{b}", bufs=1,
                                name=f"v{l}_{b}") for b in range(B)]

                # ---- B: QKV + rope, weights loaded once, both batches use ----
                wq_t = {}
                for nj in range(2):
                    wq_t[nj] = sb.tile([128, KD, 128], BF16, tag="wn", bufs=3,
                                       name=f"wq{l}_{nj}")
                    nc.sync.dma_start(wq_t[nj][:], wq_d[l, :, nj])
                wkv_t = sb.tile([128, KD, 128], BF16, tag="wn", bufs=3,
                                name=f"wkv{l}")
                nc.sync.dma_start(wkv_t[:], wkv_d[l])
                for b in range(B):
                    for nj in range(2):
                        qps = [ps.tile([128, MT], F32, tag="acc", bufs=4,
                                       name=f"qps{l}_{b}_{nj}_{i}")
                               for i in range(NBT)]
                        for ko in range(KD):
                            for mi in range(NBT):
                                nc.tensor.matmul(
                                    qps[mi][:], wq_t[nj][:, ko, :],
                                    xn[b][:, ko, ts(mi, MT)],
                                    start=(ko == 0), stop=(ko == KD - 1))
                        for mi in range(NBT):
                            rope_q(qps[mi][:], b, nj, mi, q_sb[b], f"B{l}")
                    kps = [ps.tile([64, MT], F32, tag="acc", bufs=4,
                                   name=f"kps{l}_{b}_{i}") for i in range(NBT)]
                    vps = [ps.tile([64, MT], F32, tag="acc", bufs=4,
                                   name=f"vps{l}_{b}_{i}") for i in range(NBT)]
                    for ko in range(KD):
                        for mi in range(NBT):
                            nc.tensor.matmul(
                                kps[mi][:], wkv_t[:, ko, 0:64],
                                xn[b][:, ko, ts(mi, MT)],
                                start=(ko == 0), stop=(ko == KD - 1))
                        for mi in range(NBT):
                            nc.tensor.matmul(
                                vps[mi][:], wkv_t[:, ko, 64:128],
                                xn[b][:, ko, ts(mi, MT)],
                                start=(ko == 0), stop=(ko == KD - 1))
                    for mi in range(NBT):
                        rope_k(kps[mi][:], b, mi, kk_sb[b], f"B{l}")
                        nc.vector.tensor_copy(v_sb[b][:, ts(mi, MT)], vps[mi][:])
                    nc.sync.dma_start(kk_sb[b][64:128, :], kk_sb[b][0:64, :])

                # ---- C: attention + D: AllGather o, per batch ----
                co_out = []
                for b in range(B):
                    vt_sb = sb.tile([128, 8, HD], BF16, tag=f"vt{b}", bufs=1,
                                    name=f"vt{l}_{b}")
                    for bt in range(8):
                        vp = ps.tile([128, HD], BF16, tag="sm", bufs=2,
                                     name=f"vp{l}_{b}_{bt}")
                        nc.tensor.transpose(vp[:],
                                            v_sb[b][:, bt * 128:(bt + 1) * 128],
                                            ident[0:64, 0:64])
                        nc.vector.tensor_copy(vt_sb[:, bt, :], vp[:])
                    o_sb = sb.tile([128, 2, S], BF16, tag=f"o{b}", bufs=1,
                                   name=f"o{l}_{b}")
                    for si in range(8):
                        t_len = (si + 1) * 128
                        nsc = (t_len + 511) // 512
                        for qh in range(QH):
                            qb = (qh % 2) * 64
                            lq = q_sb[b][qb:qb + 64, qh // 2,
                                         si * 128:(si + 1) * 128]
                            sc = ps.tile([128, 1024], F32, tag="sc", bufs=1,
                                         name=f"sc{l}_{b}_{si}_{qh}")
                            for tj in range(nsc):
                                tw = min(512, t_len - tj * 512)
                                nc.tensor.matmul(
                                    sc[:, tj * 512:tj * 512 + tw], lq,
                                    kk_sb[b][qb:qb + 64, tj * 512:tj * 512 + tw],
                                    start=True, stop=True)
                            nc.vector.tensor_tensor(
                                out=sc[:, si * 128:si * 128 + 128],
                                in0=sc[:, si * 128:si * 128 + 128],
                                in1=maskA[:], op=OP.add)
                            nmf = sb.tile([128, 1], F32, tag="nmf", bufs=2,
                                          name=f"nmf{l}_{b}_{si}_{qh}")
                            nc.vector.reduce_max(nmf[:], sc[:, 0:t_len],
                                                 axis=mybir.AxisListType.X,
                                                 negate=True)
                            den = sb.tile([128, 1], F32, tag="den", bufs=2,
                                          name=f"den{l}_{b}_{si}_{qh}")
                            p_t = sb.tile([128, 1024], BF16, tag="p", bufs=2,
                                          name=f"p{l}_{b}_{si}_{qh}")
                            nc.scalar.activation(
                                p_t[:, 0:t_len], sc[:, 0:t_len], AF.Exp,
                                bias=nmf[:], accum_out=den[:])
                            rden = sb.tile([128, 1], F32, tag="rden", bufs=2,
                                           name=f"rden{l}_{b}_{si}_{qh}")
                            nc.vector.reciprocal(rden[:], den[:])
                            nc.vector.tensor_scalar_mul(
                                p_t[:, 0:t_len], p_t[:, 0:t_len], rden[:])
                            ptile = sb.tile([128, 8, 128], BF16, tag="pt",
                                            bufs=2, name=f"ptile{l}_{b}_{si}_{qh}")
                            for tc in range(si + 1):
                                ptp = ps.tile([128, 128], BF16, tag="sm", bufs=2,
                                              name=f"ptp{l}_{b}_{si}_{qh}_{tc}")
                                nc.tensor.transpose(
                                    ptp[:], p_t[:, tc * 128:(tc + 1) * 128],
                                    ident[:])
                                nc.vector.tensor_copy(ptile[:, tc, :], ptp[:])
                            ov = ps.tile([64, 128], F32, tag="sm", bufs=2,
                                         name=f"ov{l}_{b}_{si}_{qh}")
                            for tc in range(si + 1):
                                nc.tensor.matmul(
                                    ov[:], vt_sb[:, tc, :], ptile[:, tc, :],
                                    start=(tc == 0), stop=(tc == si))
                            nc.vector.tensor_copy(
                                o_sb[qb:qb + 64, qh // 2,
                                     si * 128:(si + 1) * 128],
                                ov[:])
                    ci = dd.tile([256, S], BF16, tag=f"co_in{b}", bufs=2,
                                 name=f"co_in{l}_{b}")
                    nc.sync.dma_start(
                        ci.rearrange("(ko ki) m -> ki ko m", ki=128)[:], o_sb[:])
                    co = dd.tile([DIM, S], BF16, tag=f"co_out{b}", bufs=2,
                                 addr_space="Shared", name=f"co_out{l}_{b}")
                    nc.gpsimd.collective_compute(
                        "AllGather", OP.bypass, replica_groups=RG,
                        ins=[ci[:].opt()], outs=[co[:].opt()])
                    co_out.append(co)

                # ---- E: wo GEMM + residual, per batch (weights streamed)
                h1_sb, ch1_out = [], []
                for b in range(B):
                    wops = [ps.tile([128, MT], F32, tag="acc", bufs=4,
                                    name=f"wops{l}_{b}_{i}") for i in range(4)]
                    co_r = co_out[b].rearrange("(ko ki) m -> ki ko m", ki=128)
                    for kb in range(4):
                        wo_t = sb.tile([128, 4, 256], BF16, tag="wk2", bufs=3,
                                       name=f"wo{l}_{b}_{kb}")
                        nc.sync.dma_start(wo_t[:],
                                          wo_d[l, :, kb * 4:(kb + 1) * 4, :])
                        for j in range(4):
                            ko = kb * 4 + j
                            ot = sb.tile([128, S], BF16, tag="kst", bufs=3,
                                         name=f"ot{l}_{b}_{ko}")
                            nc.sync.dma_start(ot[:], co_r[:, ko, :])
                            for njj in range(2):
                                for mi in range(NBT):
                                    nc.tensor.matmul(
                                        wops[mi * 2 + njj][:],
                                        wo_t[:, j, njj * 128:(njj + 1) * 128],
                                        ot[:, ts(mi, MT)],
                                        start=(ko == 0), stop=(ko == KD - 1))
                    h1 = sb.tile([128, 2, S], F32, tag=f"h1_{b}", bufs=1,
                                 name=f"h1_{l}_{b}")
                    ci = dd.tile([257, S], BF16, tag=f"ch1_in{b}", bufs=2,
                                 name=f"ch1_in{l}_{b}")
                    emit_resid(l, b, "1", h1, wops, cur_h[b], ci)
                    co = dd.tile([NCORES * 257, S], BF16, tag=f"ch1_out{b}",
                                 bufs=2, addr_space="Shared",
                                 name=f"ch1_out{l}_{b}")
                    nc.gpsimd.collective_compute(
                        "AllGather", OP.bypass, replica_groups=RG,
                        ins=[ci[:].opt()], outs=[co[:].opt()])
                    h1_sb.append(h1)
                    ch1_out.append(co)

                # ---- G: norm2 per batch ----
                for b in range(B):
                    norm_from_ag(b, ch1_out[b], f"G{l}")

                # ---- H: w1/w3 GEMM (batches interleaved per nj block) ----
                ca_in = [dd.tile([FFN_PAD, S], BF16, tag=f"ca_in{b}", bufs=2,
                                 name=f"ca_in{l}_{b}") for b in range(B)]
                ca_out = []
                for b in range(B):
                    for nj in range(6):
                        w1_t = sb.tile([128, KD, 128], BF16, tag="wn", bufs=3,
                                       name=f"w1_{l}_{b}_{nj}")
                        nc.sync.dma_start(w1_t[:], w1_d[l, :, nj])
                        w3_t = sb.tile([128, KD, 128], BF16, tag="wn", bufs=3,
                                       name=f"w3_{l}_{b}_{nj}")
                        nc.sync.dma_start(w3_t[:], w3_d[l, :, nj])
                        aps = [ps.tile([128, MT], F32, tag="acc", bufs=4,
                                       name=f"aps{l}_{b}_{nj}_{i}")
                               for i in range(NBT)]
                        bps = [ps.tile([128, MT], F32, tag="acc", bufs=4,
                                       name=f"bps{l}_{b}_{nj}_{i}")
                               for i in range(NBT)]
                        for ko in range(KD):
                            for mi in range(NBT):
                                nc.tensor.matmul(
                                    aps[mi][:], w1_t[:, ko, :],
                                    xn[b][:, ko, ts(mi, MT)],
                                    start=(ko == 0), stop=(ko == KD - 1))
                            for mi in range(NBT):
                                nc.tensor.matmul(
                                    bps[mi][:], w3_t[:, ko, :],
                                    xn[b][:, ko, ts(mi, MT)],
                                    start=(ko == 0), stop=(ko == KD - 1))
                        ca_r = ca_in[b].rearrange("(nj ki) m -> ki nj m", ki=128)
                        for mi in range(NBT):
                            sil = sb.tile([128, MT], BF16, tag="sil", bufs=2,
                                          name=f"sil{l}_{b}_{nj}_{mi}")
                            nc.scalar.activation(sil[:], aps[mi][:], AF.Silu)
                            at = sb.tile([128, MT], BF16, tag="at", bufs=2,
                                         name=f"at{l}_{b}_{nj}_{mi}")
                            nc.vector.tensor_tensor(out=at[:], in0=bps[mi][:],
                                                    in1=sil[:], op=OP.mult)
                            nc.sync.dma_start(ca_r[:, nj, ts(mi, MT)], at[:])
                    if nj == 5:
                        co = dd.tile([HID_PAD, S], BF16, tag=f"ca_out{b}",
                                     bufs=2, addr_space="Shared",
                                     name=f"ca_out{l}_{b}")
                        nc.gpsimd.collective_compute(
                            "AllGather", OP.bypass, replica_groups=RG,
                            ins=[ca_in[b][:].opt()], outs=[co[:].opt()])
                        ca_out.append(co)

                # ---- J: w2 GEMM per batch + residual ----
                new_h = [sb.tile([128, 2, S], F32, tag=f"h2_{b}", bufs=1,
                                 name=f"h2_{l}_{b}") for b in range(B)]
                for b in range(B):
                    w2ps = [ps.tile([128, MT], F32, tag="acc", bufs=4,
                                    name=f"w2ps{l}_{b}_{i}") for i in range(4)]
                    ca_r = ca_out[b].rearrange("(ko ki) m -> ki ko m", ki=128)
                    for kb in range(W2KD // 4):
                        w2_t = sb.tile([128, 4, 256], BF16, tag="wk2", bufs=3,
                                       name=f"w2_{l}_{b}_{kb}")
                        nc.sync.dma_start(w2_t[:],
                                          w2_d[l, :, kb * 4:(kb + 1) * 4, :])
                        for j in range(4):
                            ko = kb * 4 + j
                            at2 = sb.tile([128, S], BF16, tag="kst", bufs=3,
                                          name=f"at2_{l}_{b}_{ko}")
                            nc.sync.dma_start(at2[:], ca_r[:, ko, :])
                            for njj in range(2):
                                for mi in range(NBT):
                                    nc.tensor.matmul(
                                        w2ps[mi * 2 + njj][:],
                                        w2_t[:, j, njj * 128:(njj + 1) * 128],
                                        at2[:, ts(mi, MT)],
                                        start=(ko == 0), stop=(ko == W2KD - 1))
                    ci = dd.tile([257, S], BF16, tag=f"ch2_in{b}", bufs=2,
                                 name=f"ch2_in{l}_{b}")
                    emit_resid(l, b, "2", new_h[b], w2ps, h1_sb[b], ci)
                    co = dd.tile([NCORES * 257, S], BF16, tag=f"ch2_out{b}",
                                 bufs=2, addr_space="Shared",
                                 name=f"ch2_out{l}_{b}")
                    nc.gpsimd.collective_compute(
                        "AllGather", OP.bypass, replica_groups=RG,
                        ins=[ci[:].opt()], outs=[co[:].opt()])
                    # next layer's norm1 (or the final norm) for this batch
                    norm_from_ag(b, co, f"K{l}")
                cur_h = new_h

            # ============ output head ============
            for nf in range(NF):
                wt = sb.tile([128, KD, 128], BF16, tag="wn", bufs=3,
                             name=f"wout{nf}")
                nc.sync.dma_start(wt[:], wout_d[:, nf])
                hps = [ps.tile([128, MT], F32, tag="acc", bufs=4,
                               name=f"hps{nf}_{i}") for i in range(4)]
                for ko in range(KD):
                    for gmi in range(4):
                        nc.tensor.matmul(
                            hps[gmi][:], wt[:, ko, :],
                            xn[gmi // 2][:, ko, ts(gmi % 2, MT)],
                            start=(ko == 0), stop=(ko == KD - 1))
                for gmi in range(4):
                    lg = sb.tile([128, MT], F32, tag="lg", bufs=1,
                                 name=f"lg{nf}_{gmi}")
                    nc.scalar.copy(lg[:], hps[gmi][:])
                    nc.sync.dma_start(
                        logT_d[nf * 128:(nf + 1) * 128, ts(gmi, MT)], lg[:])

    nc.compile()
    return nc


_ROPE_PERM = np.concatenate([np.arange(0, HD, 2), np.arange(1, HD, 2)])


def _perm_heads(w):
    """Permute rope pairs within each 64-col head block. w: [K, n_heads*64]."""
    K, N = w.shape
    return np.ascontiguousarray(
        w.reshape(K, N // HD, HD)[:, :, _ROPE_PERM].reshape(K, N))


def _pack_k(w):
    """[K, N] -> [128, K//128, N] with feature f = ko*128 + ki."""
    K, N = w.shape
    return np.ascontiguousarray(w.reshape(K // 128, 128, N).transpose(1, 0, 2))


def _pack_n(w, nblk=128):
    """[K, N] -> [128, N//nblk, K//128, nblk]."""
    K, N = w.shape
    x = w.reshape(K // 128, 128, N // nblk, nblk)
    return np.ascontiguousarray(x.transpose(1, 2, 0, 3))


def _prep_inputs(inputs):
    f32 = np.float32
    tokens = np.asarray(inputs["tokens"]).astype(np.int64).reshape(-1)
    emb = np.asarray(inputs["emb_W"], dtype=f32)
    wq = np.asarray(inputs["wq"], dtype=f32)
    wk = np.asarray(inputs["wk"], dtype=f32)
    wv = np.asarray(inputs["wv"], dtype=f32)
    wo = np.asarray(inputs["wo"], dtype=f32)
    w1 = np.asarray(inputs["w1"], dtype=f32)
    w2 = np.asarray(inputs["w2"], dtype=f32)
    w3 = np.asarray(inputs["w3"], dtype=f32)
    an = np.asarray(inputs["attn_norm_w"], dtype=f32)
    fn = np.asarray(inputs["ffn_norm_w"], dtype=f32)
    nw = np.asarray(inputs["norm_w"], dtype=f32)
    outw = np.asarray(inputs["out_W"], dtype=f32)
    cos = np.asarray(inputs["freqs_cos"], dtype=f32)
    sin = np.asarray(inputs["freqs_sin"], dtype=f32)

    h0T = np.ascontiguousarray(emb[tokens].T)          # [2048, 2048] f32
    hn0 = _pack_k(h0T).astype(nbf16)                   # [128, 16, 2048]

    ct = np.ascontiguousarray(cos.T).astype(f32)       # [32, 1024]
    st = np.ascontiguousarray(sin.T).astype(f32)
    scale = np.float32(1.0 / np.sqrt(HD))
    cq, sq = scale * ct, scale * st
    TA = np.concatenate([cq, cq, cq, cq], axis=0).astype(nbf16)
    TBt = np.concatenate([sq, sq, sq, sq], axis=0).astype(nbf16)
    TKt = np.concatenate([ct, ct, st, st], axis=0).astype(nbf16)
    ropetab = np.stack([TA, TBt, TKt])                 # [3, 128, 1024]

    ssq0 = (h0T.astype(np.float64) ** 2).sum(axis=0).astype(f32)[None, :]

    tri = np.tril(np.ones((128, 128), dtype=bool))
    maskA = np.where(tri, 0.0, -1e30).astype(f32)

    wq_f = wq * an[:, :, None]
    wk_f = wk * an[:, :, None]
    wv_f = wv * an[:, :, None]
    w1_f = w1 * fn[:, :, None]
    w3_f = w3 * fn[:, :, None]
    outw_f = outw * nw[:, None]

    in_maps = []
    for r in range(NCORES):
        m = {
            "hn0": hn0,
            "h0r": np.ascontiguousarray(
                h0T[r * 256:(r + 1) * 256].reshape(2, 128, M).transpose(1, 0, 2)),
            "ropetab": ropetab,
            "maskA": maskA,
            "ssq0": ssq0,
        }
        wq_l, wkv_l, wo_l, w1_l, w3_l, w2_l = [], [], [], [], [], []
        for l in range(L):
            wq_r = _perm_heads(wq_f[l][:, r * 256:(r + 1) * 256])
            wq_l.append(_pack_n(wq_r.astype(nbf16)))
            wk_r = _perm_heads(wk_f[l][:, r * 64:(r + 1) * 64])
            wv_r = wv_f[l][:, r * 64:(r + 1) * 64]
            wkv_l.append(_pack_n(
                np.concatenate([wk_r, wv_r], axis=1).astype(nbf16))[:, 0])
            wo_l.append(_pack_k(
                wo[l][:, r * 256:(r + 1) * 256].astype(nbf16)))
            w1_r = np.zeros((DIM, FFN_PAD), dtype=f32)
            w1_r[:, :FFN_N] = w1_f[l][:, r * FFN_N:(r + 1) * FFN_N]
            w1_l.append(_pack_n(w1_r.astype(nbf16)))
            w3_r = np.zeros((DIM, FFN_PAD), dtype=f32)
            w3_r[:, :FFN_N] = w3_f[l][:, r * FFN_N:(r + 1) * FFN_N]
            w3_l.append(_pack_n(w3_r.astype(nbf16)))
            w2_r = np.zeros((HID_PAD, 256), dtype=f32)
            for jr in range(NCORES):
                w2_r[jr * FFN_PAD:jr * FFN_PAD + FFN_N] = \
                    w2[l][jr * FFN_N:(jr + 1) * FFN_N, r * 256:(r + 1) * 256]
            w2_l.append(_pack_k(w2_r.astype(nbf16)))
        m["wq"] = np.stack(wq_l)
        m["wkv"] = np.stack(wkv_l)
        m["wo"] = np.stack(wo_l)
        m["w1"] = np.stack(w1_l)
        m["w3"] = np.stack(w3_l)
        m["w2"] = np.stack(w2_l)
        wout_r = np.zeros((DIM, OUT_PAD), dtype=f32)
        wout_r[:, :OUT_N] = outw_f[:, r * OUT_N:(r + 1) * OUT_N]
        m["wout"] = _pack_n(wout_r.astype(nbf16))
        in_maps.append(m)
    return in_maps


_NC_CACHE = {}


def _get_nc():
    if "nc" not in _NC_CACHE:
        _NC_CACHE["nc"] = _build()
    return _NC_CACHE["nc"]


def run(inputs, trace=False):
    nc = _get_nc()
    in_maps = _prep_inputs(inputs)
    res = run_bass_kernel_spmd(nc, in_maps, core_ids=list(range(NCORES)),
                               trace=trace)
    logits = np.empty((M, VOCAB_TOTAL), dtype=np.float32)
    for r in range(NCORES):
        lt = res.results[r]["logT"]
        logits[:, r * OUT_N:(r + 1) * OUT_N] = lt[:OUT_N].T
    return logits.reshape(B, S, VOCAB_TOTAL), res


def kernel(**inputs):
    out, _ = run(inputs, trace=False)
    return out


# revision 20
# speedup vs baseline: 1.3357x; 1.0571x over previous
"""Llama forward-pass Trainium2 kernel: 8-core tensor-parallel (column-sharded
weights + AllGather between GEMM groups), bf16 matmuls with fp32 accumulation.

The two batch rows are independent streams: every phase and collective is
split per batch and emitted phase-skewed so one batch's PE work hides the
other's AllGather / softmax / norm latency.

Self-contained: hardcodes all shapes. kernel(**inputs) -> logits [2,1024,32000].
"""
import numpy as np
import ml_dtypes

import concourse.bass as bass
import concourse.bacc as bacc
import concourse.mybir as mybir
import concourse.tile as tile
from concourse.bass import ts
from concourse.bass_utils import run_bass_kernel_spmd
from concourse.masks import make_identity

AF = mybir.ActivationFunctionType
OP = mybir.AluOpType
BF16 = mybir.dt.bfloat16
F32 = mybir.dt.float32
nbf16 = ml_dtypes.bfloat16

NCORES = 8
L = 4
DIM = 2048
KD = DIM // 128            # 16
HD = 64
QH = 4                     # q heads per core
B, S = 2, 1024
M = B * S                  # 2048 tokens
MT = 512
NBT = 2                    # m tiles of 512 per batch
HIDDEN = 5632
FFN_N = 704                # per-core ffn cols
FFN_PAD = 768              # padded to 6 x 128
HID_PAD = FFN_PAD * NCORES # 6144
W2KD = HID_PAD // 128      # 48
OUT_N = 4000
OUT_PAD = 4096
NF = OUT_PAD // 128        # 32
EPS = 1e-5
RG = [list(range(NCORES))]
VOCAB_TOTAL = 32000


def _build():
    nc = bacc.Bacc("TRN2", target_bir_lowering=False, debug=False,
                   num_devices=NCORES)

    hn0_d = nc.dram_tensor("hn0", [128, KD, M], BF16, kind="ExternalInput")
    h0r_d = nc.dram_tensor("h0r", [128, 2, M], F32, kind="ExternalInput")
    rt_d = nc.dram_tensor("ropetab", [3, 128, S], BF16, kind="ExternalInput")
    mask_d = nc.dram_tensor("maskA", [128, 128], F32, kind="ExternalInput")
    ssq0_d = nc.dram_tensor("ssq0", [1, M], F32, kind="ExternalInput")
    wq_d = nc.dram_tensor("wq", [L, 128, 2, KD, 128], BF16, kind="ExternalInput")
    wkv_d = nc.dram_tensor("wkv", [L, 128, KD, 128], BF16, kind="ExternalInput")
    wo_d = nc.dram_tensor("wo", [L, 128, KD, 256], BF16, kind="ExternalInput")
    w1_d = nc.dram_tensor("w1", [L, 128, 6, KD, 128], BF16, kind="ExternalInput")
    w3_d = nc.dram_tensor("w3", [L, 128, 6, KD, 128], BF16, kind="ExternalInput")
    w2_d = nc.dram_tensor("w2", [L, 128, W2KD, 256], BF16, kind="ExternalInput")
    wout_d = nc.dram_tensor("wout", [128, NF, KD, 128], BF16, kind="ExternalInput")
    logT_d = nc.dram_tensor("logT", [OUT_PAD, M], F32, kind="ExternalOutput")

    with tile.TileContext(nc) as tc:
        with (
            tc.tile_pool(name="cst", bufs=1) as cst,
            tc.tile_pool(name="sb", bufs=2) as sb,
            tc.tile_pool(name="ps", bufs=1, space="PSUM") as ps,
            tc.tile_pool(name="dd", bufs=2, space="DRAM") as dd,
        ):
            # ---- constants ----
            TA = cst.tile([128, S], BF16)
            nc.sync.dma_start(TA[:], rt_d[0])
            TB = cst.tile([128, S], BF16)
            nc.sync.dma_start(TB[:], rt_d[1])
            TK = cst.tile([128, S], BF16)
            nc.sync.dma_start(TK[:], rt_d[2])
            maskA = cst.tile([128, 128], F32)
            nc.sync.dma_start(maskA[:], mask_d[:])
            ident = cst.tile([128, 128], BF16)
            make_identity(nc, ident[:])
            ones_sq = cst.tile([128, 1], BF16)
            nc.vector.memset(ones_sq[:], 1.0)
            ones_bc = cst.tile([1, 128], F32)
            nc.vector.memset(ones_bc[:], 1.0)
            eps_t = cst.tile([1, 1], F32)
            nc.vector.memset(eps_t[:], EPS)

            # ---- persistent per-batch activations ----
            xn = [[sb.tile([128, KD, MT], BF16, tag=f"xn{b}_{mi}", bufs=1,
                           name=f"xn_{b}_{mi}") for mi in range(NBT)]
                  for b in range(B)]
            cur_h = [sb.tile([128, 2, S], F32, tag=f"h2_{b}", bufs=1,
                             name=f"h0_{b}") for b in range(B)]
            for b in range(B):
                nc.sync.dma_start(cur_h[b][:], h0r_d[:, :, ts(b, S)])

            def _norm_tail(b, mi, hn_t, ssq_ap, is_psum, pfx):
                """ssq_ap: [1, MT] total sum-of-squares (psum or sbuf)."""
                sqv = sb.tile([1, MT], F32, tag="sqv", bufs=1,
                              name=f"{pfx}sqv{b}_{mi}")
                nc.scalar.activation(sqv[:], ssq_ap, AF.Sqrt,
                                     bias=eps_t[:], scale=1.0 / DIM)
                rstd = sb.tile([1, MT], F32, tag="rstd", bufs=1,
                               name=f"{pfx}rstd{b}_{mi}")
                nc.vector.reciprocal(rstd[:], sqv[:])
                bc = ps.tile([128, MT], F32, tag="sm", bufs=2,
                             name=f"{pfx}bc{b}_{mi}")
                nc.tensor.matmul(bc[:], ones_bc[:], rstd[:],
                                 start=True, stop=True)
                bcs = sb.tile([128, MT], F32, tag="bcs", bufs=1,
                              name=f"{pfx}bcs{b}_{mi}")
                nc.scalar.copy(bcs[:], bc[:])
                for ko in range(KD):
                    eng = nc.vector if ko % 2 == 0 else nc.gpsimd
                    eng.tensor_tensor(
                        out=xn[b][mi][:, ko, :], in0=hn_t[:, ko, :],
                        in1=bcs[:], op=OP.mult)

            def norm_l0(b, pfx):
                """Layer-0 norm: ssq precomputed on host."""
                for mi in range(NBT):
                    hn_t = sb.tile([128, KD, MT], BF16, tag="hn", bufs=1,
                                   name=f"{pfx}hn{b}_{mi}")
                    nc.sync.dma_start(hn_t[:],
                                      hn0_d[:, :, b * S + mi * MT:
                                            b * S + (mi + 1) * MT])
                    s0 = sb.tile([1, MT], F32, tag="s0", bufs=1,
                                 name=f"{pfx}s0_{b}_{mi}")
                    nc.sync.dma_start(s0[:],
                                      ssq0_d[:, b * S + mi * MT:
                                             b * S + (mi + 1) * MT])
                    _norm_tail(b, mi, hn_t, s0[:], False, pfx)

            def norm_from_ag(b, co, pfx):
                """co: [NCORES*257, S] AG output carrying h rows + ssq row."""
                co3 = co.rearrange("(r x) m -> r x m", x=257)
                ssqp = sb.tile([8, S], BF16, tag="ssqp", bufs=1,
                               name=f"{pfx}ssqp{b}")
                nc.sync.dma_start(ssqp[:], co3[:, 256, :])
                hview = co3[:, 0:256, :].rearrange(
                    "r (ko ki) m -> ki r ko m", ki=128)
                for mi in range(NBT):
                    hn_t = sb.tile([128, KD, MT], BF16, tag="hn", bufs=1,
                                   name=f"{pfx}hn{b}_{mi}")
                    hn4 = hn_t[:].rearrange("p (r ko) m -> p r ko m", ko=2)
                    for ko_l in range(2):
                        nc.sync.dma_start(hn4[:, :, ko_l, :],
                                          hview[:, :, ko_l, ts(mi, MT)])
                    ssq = ps.tile([1, MT], F32, tag="sm", bufs=2,
                                  name=f"{pfx}ssq{b}_{mi}")
                    nc.tensor.matmul(ssq[:], ones_sq[0:8, :],
                                     ssqp[:, ts(mi, MT)],
                                     start=True, stop=True)
                    _norm_tail(b, mi, hn_t, ssq[:], True, pfx)

            def rope_q(psrc, b, nj, mi, q_t, pfx):
                """psrc: [128, MT] psum with 2 heads [a;b][a;b]. Writes q_t.
                uc (SBUF) = x*cos, us (PSUM) = x*sin; combines mix one SBUF +
                one PSUM input so partition bases are unconstrained."""
                uc = sb.tile([128, MT], BF16, tag="ru", bufs=2,
                             name=f"{pfx}uc{b}_{nj}_{mi}")
                us = ps.tile([128, MT], F32, tag="sm", bufs=2,
                             name=f"{pfx}us{b}_{nj}_{mi}")
                tsl = ts(mi, MT)
                nc.vector.tensor_tensor(out=uc[:], in0=psrc[:], in1=TA[:, tsl],
                                        op=OP.mult)
                nc.vector.tensor_tensor(out=us[:], in0=psrc[:], in1=TB[:, tsl],
                                        op=OP.mult)
                for hh in range(2):
                    base = hh * 64
                    nc.vector.tensor_tensor(
                        out=q_t[base:base + 32, nj, tsl],
                        in0=uc[base:base + 32, :], in1=us[base + 32:base + 64, :],
                        op=OP.subtract)
                    nc.vector.tensor_tensor(
                        out=q_t[base + 32:base + 64, nj, tsl],
                        in0=us[base:base + 32, :], in1=uc[base + 32:base + 64, :],
                        op=OP.add)

            def rope_k(psrc, b, mi, kk_t, pfx):
                uc = sb.tile([64, MT], BF16, tag="ruk", bufs=2,
                             name=f"{pfx}uck{b}_{mi}")
                us = ps.tile([64, MT], F32, tag="sm", bufs=2,
                             name=f"{pfx}usk{b}_{mi}")
                tsl = ts(mi, MT)
                nc.vector.tensor_tensor(out=uc[:], in0=psrc[:], in1=TK[0:64, tsl],
                                        op=OP.mult)
                nc.vector.tensor_tensor(out=us[:], in0=psrc[:], in1=TK[64:128, tsl],
                                        op=OP.mult)
                nc.vector.tensor_tensor(out=kk_t[0:32, tsl], in0=uc[0:32, :],
                                        in1=us[32:64, :], op=OP.subtract)
                nc.vector.tensor_tensor(out=kk_t[32:64, tsl], in0=us[0:32, :],
                                        in1=uc[32:64, :], op=OP.add)

            def emit_resid(l, b, which, hdst, psums, hres, ci):
                """hdst = psums + hres per (mi,njj); write bf16 rows + local
                ssq partial (row 256) into the AG input tile ci [257, S]."""
                ci_r = ci[0:256, :].rearrange("(ko ki) m -> ki ko m", ki=128)
                for mi in range(NBT):
                    for njj in range(2):
                        nc.vector.tensor_tensor(
                            out=hdst[:, njj, ts(mi, MT)],
                            in0=psums[mi * 2 + njj][:],
                            in1=hres[:, njj, ts(mi, MT)], op=OP.add)
                        hb = sb.tile([128, MT], BF16, tag="hb", bufs=2,
                                     name=f"hb{which}_{l}_{b}_{mi}_{njj}")
                        nc.gpsimd.tensor_copy(hb[:], hdst[:, njj, ts(mi, MT)])
                        nc.sync.dma_start(ci_r[:, njj, ts(mi, MT)], hb[:])
                    sq_t = sb.tile([128, 2, MT], BF16, tag="sq", bufs=2,
                                   name=f"sqr{which}_{l}_{b}_{mi}")
                    nc.scalar.square(sq_t[:], hdst[:, :, ts(mi, MT)])
                    sps = ps.tile([1, MT], F32, tag="sm", bufs=2,
                                  name=f"sps{which}_{l}_{b}_{mi}")
                    for j in range(2):
                        nc.tensor.matmul(sps[:], ones_sq[:], sq_t[:, j, :],
                                         start=(j == 0), stop=(j == 1))
                    sqb = sb.tile([1, MT], BF16, tag="sqb", bufs=2,
                                  name=f"sqb{which}_{l}_{b}_{mi}")
                    nc.vector.tensor_copy(sqb[:], sps[:])
                    nc.sync.dma_start(ci[256:257, ts(mi, MT)], sqb[:])

            for l in range(L):
                # ---- A: norm1 per batch ----
                if l == 0:
                    for b in range(B):
                        norm_l0(b, f"A{l}")
                # (l>0: norm over ch2_out emitted at end of previous layer)

                q_sb = [sb.tile([128, 2, S], BF16, tag=f"q{b}", bufs=1,
                                name=f"q{l}_{b}") for b in range(B)]
                kk_sb = [sb.tile([128, S], BF16, tag=f"k{b}", bufs=1,
                                 name=f"kk{l}_{b}") for b in range(B)]
                v_sb = [sb.tile([64, S], BF16, tag=f"v{b}", bufs=1,
                                name=f"v{l}_{b}") for b in range(B)]

                # ---- B: QKV + rope, weights loaded once, both batches use ----
                wq_t = {}
                for nj in range(2):
                    wq_t[nj] = sb.tile([128, KD, 128], BF16, tag="wn", bufs=3,
                                       name=f"wq{l}_{nj}")
                    nc.sync.dma_start(wq_t[nj][:], wq_d[l, :, nj])
                wkv_t = sb.tile([128, KD, 128], BF16, tag="wn", bufs=3,
                                name=f"wkv{l}")
                nc.sync.dma_start(wkv_t[:], wkv_d[l])
                for b in range(B):
                    for nj in range(2):
                        qps = [ps.tile([128, MT], F32, tag="acc", bufs=4,
                                       name=f"qps{l}_{b}_{nj}_{i}")
                               for i in range(NBT)]
                        for ko in range(KD):
                            for mi in range(NBT):
                                nc.tensor.matmul(
                                    qps[mi][:], wq_t[nj][:, ko, :],
                                    xn[b][mi][:, ko, :],
                                    start=(ko == 0), stop=(ko == KD - 1))
                        for mi in range(NBT):
                            rope_q(qps[mi][:], b, nj, mi, q_sb[b], f"B{l}")
                    kps = [ps.tile([64, MT], F32, tag="acc", bufs=4,
                                   name=f"kps{l}_{b}_{i}") for i in range(NBT)]
                    vps = [ps.tile([64, MT], F32, tag="acc", bufs=4,
                                   name=f"vps{l}_{b}_{i}") for i in range(NBT)]
                    for ko in range(KD):
                        for mi in range(NBT):
                            nc.tensor.matmul(
                                kps[mi][:], wkv_t[:, ko, 0:64],
                                xn[b][mi][:, ko, :],
                                start=(ko == 0), stop=(ko == KD - 1))
                        for mi in range(NBT):
                            nc.tensor.matmul(
                                vps[mi][:], wkv_t[:, ko, 64:128],
                                xn[b][mi][:, ko, :],
                                start=(ko == 0), stop=(ko == KD - 1))
                    for mi in range(NBT):
                        rope_k(kps[mi][:], b, mi, kk_sb[b], f"B{l}")
                        nc.vector.tensor_copy(v_sb[b][:, ts(mi, MT)], vps[mi][:])
                    nc.sync.dma_start(kk_sb[b][64:128, :], kk_sb[b][0:64, :])

                # ---- C: attention + D: AllGather o, per batch ----
                co_out = []
                for b in range(B):
                    vt_sb = sb.tile([128, 8, HD], BF16, tag=f"vt{b}", bufs=1,
                                    name=f"vt{l}_{b}")
                    for bt in range(8):
                        vp = ps.tile([128, HD], BF16, tag="sm", bufs=2,
                                     name=f"vp{l}_{b}_{bt}")
                        nc.tensor.transpose(vp[:],
                                            v_sb[b][:, bt * 128:(bt + 1) * 128],
                                            ident[0:64, 0:64])
                        nc.vector.tensor_copy(vt_sb[:, bt, :], vp[:])
                    o_sb = sb.tile([128, 2, S], BF16, tag=f"o{b}", bufs=1,
                                   name=f"o{l}_{b}")
                    for si in range(8):
                        t_len = (si + 1) * 128
                        nsc = (t_len + 511) // 512
                        for qh in range(QH):
                            qb = (qh % 2) * 64
                            lq = q_sb[b][qb:qb + 64, qh // 2,
                                         si * 128:(si + 1) * 128]
                            sc = ps.tile([128, 1024], F32, tag="sc", bufs=1,
                                         name=f"sc{l}_{b}_{si}_{qh}")
                            for tj in range(nsc):
                                tw = min(512, t_len - tj * 512)
                                nc.tensor.matmul(
                                    sc[:, tj * 512:tj * 512 + tw], lq,
                                    kk_sb[b][qb:qb + 64, tj * 512:tj * 512 + tw],
                                    start=True, stop=True)
                            nc.vector.tensor_tensor(
                                out=sc[:, si * 128:si * 128 + 128],
                                in0=sc[:, si * 128:si * 128 + 128],
                                in1=maskA[:], op=OP.add)
                            nmf = sb.tile([128, 1], F32, tag="nmf", bufs=2,
                                          name=f"nmf{l}_{b}_{si}_{qh}")
                            nc.vector.reduce_max(nmf[:], sc[:, 0:t_len],
                                                 axis=mybir.AxisListType.X,
                                                 negate=True)
                            den = sb.tile([128, 1], F32, tag="den", bufs=2,
                                          name=f"den{l}_{b}_{si}_{qh}")
                            p_t = sb.tile([128, 1024], BF16, tag="p", bufs=2,
                                          name=f"p{l}_{b}_{si}_{qh}")
                            nc.scalar.activation(
                                p_t[:, 0:t_len], sc[:, 0:t_len], AF.Exp,
                                bias=nmf[:], accum_out=den[:])
                            rden = sb.tile([128, 1], F32, tag="rden", bufs=2,
                                           name=f"rden{l}_{b}_{si}_{qh}")
                            nc.vector.reciprocal(rden[:], den[:])
                            nc.vector.tensor_scalar_mul(
                                p_t[:, 0:t_len], p_t[:, 0:t_len], rden[:])
                            ptile = sb.tile([128, 8, 128], BF16, tag="pt",
                                            bufs=2, name=f"ptile{l}_{b}_{si}_{qh}")
                            for tc in range(si + 1):
                                ptp = ps.tile([128, 128], BF16, tag="sm", bufs=2,
                                              name=f"ptp{l}_{b}_{si}_{qh}_{tc}")
                                nc.tensor.transpose(
                                    ptp[:], p_t[:, tc * 128:(tc + 1) * 128],
                                    ident[:])
                                nc.vector.tensor_copy(ptile[:, tc, :], ptp[:])
                            ov = ps.tile([64, 128], F32, tag="sm", bufs=2,
                                         name=f"ov{l}_{b}_{si}_{qh}")
                            for tc in range(si + 1):
                                nc.tensor.matmul(
                                    ov[:], vt_sb[:, tc, :], ptile[:, tc, :],
                                    start=(tc == 0), stop=(tc == si))
                            nc.vector.tensor_copy(
                                o_sb[qb:qb + 64, qh // 2,
                                     si * 128:(si + 1) * 128],
                                ov[:])
                    ci = dd.tile([256, S], BF16, tag=f"co_in{b}", bufs=2,
                                 name=f"co_in{l}_{b}")
                    nc.sync.dma_start(
                        ci.rearrange("(ko ki) m -> ki ko m", ki=128)[:], o_sb[:])
                    co = dd.tile([DIM, S], BF16, tag=f"co_out{b}", bufs=2,
                                 addr_space="Shared", name=f"co_out{l}_{b}")
                    nc.gpsimd.collective_compute(
                        "AllGather", OP.bypass, replica_groups=RG,
                        ins=[ci[:].opt()], outs=[co[:].opt()])
                    co_out.append(co)

                # ---- E: wo GEMM + residual, per batch (weights streamed)
                h1_sb, ch1_out = [], []
                for b in range(B):
                    wops = [ps.tile([128, MT], F32, tag="acc", bufs=4,
                                    name=f"wops{l}_{b}_{i}") for i in range(4)]
                    co_r = co_out[b].rearrange("(ko ki) m -> ki ko m", ki=128)
                    for kb in range(4):
                        wo_t = sb.tile([128, 4, 256], BF16, tag="wk2", bufs=3,
                                       name=f"wo{l}_{b}_{kb}")
                        nc.sync.dma_start(wo_t[:],
                                          wo_d[l, :, kb * 4:(kb + 1) * 4, :])
                        for j in range(4):
                            ko = kb * 4 + j
                            ot = sb.tile([128, S], BF16, tag="kst", bufs=3,
                                         name=f"ot{l}_{b}_{ko}")
                            nc.sync.dma_start(ot[:], co_r[:, ko, :])
                            for njj in range(2):
                                for mi in range(NBT):
                                    nc.tensor.matmul(
                                        wops[mi * 2 + njj][:],
                                        wo_t[:, j, njj * 128:(njj + 1) * 128],
                                        ot[:, ts(mi, MT)],
                                        start=(ko == 0), stop=(ko == KD - 1))
                    h1 = sb.tile([128, 2, S], F32, tag=f"h1_{b}", bufs=1,
                                 name=f"h1_{l}_{b}")
                    ci = dd.tile([257, S], BF16, tag=f"ch1_in{b}", bufs=2,
                                 name=f"ch1_in{l}_{b}")
                    emit_resid(l, b, "1", h1, wops, cur_h[b], ci)
                    co = dd.tile([NCORES * 257, S], BF16, tag=f"ch1_out{b}",
                                 bufs=2, addr_space="Shared",
                                 name=f"ch1_out{l}_{b}")
                    nc.gpsimd.collective_compute(
                        "AllGather", OP.bypass, replica_groups=RG,
                        ins=[ci[:].opt()], outs=[co[:].opt()])
                    h1_sb.append(h1)
                    ch1_out.append(co)

                # ---- G: norm2 per batch ----
                for b in range(B):
                    norm_from_ag(b, ch1_out[b], f"G{l}")

                # ---- H: w1/w3 GEMM (batches interleaved per nj block) ----
                ca_in = [dd.tile([FFN_PAD, S], BF16, tag=f"ca_in{b}", bufs=2,
                                 name=f"ca_in{l}_{b}") for b in range(B)]
                ca_out = []
                for b in range(B):
                    for nj in range(6):
                        w1_t = sb.tile([128, KD, 128], BF16, tag="wn", bufs=3,
                                       name=f"w1_{l}_{b}_{nj}")
                        nc.sync.dma_start(w1_t[:], w1_d[l, :, nj])
                        w3_t = sb.tile([128, KD, 128], BF16, tag="wn", bufs=3,
                                       name=f"w3_{l}_{b}_{nj}")
                        nc.sync.dma_start(w3_t[:], w3_d[l, :, nj])
                        aps = [ps.tile([128, MT], F32, tag="acc", bufs=4,
                                       name=f"aps{l}_{b}_{nj}_{i}")
                               for i in range(NBT)]
                        bps = [ps.tile([128, MT], F32, tag="acc", bufs=4,
                                       name=f"bps{l}_{b}_{nj}_{i}")
                               for i in range(NBT)]
                        for ko in range(KD):
                            for mi in range(NBT):
                                nc.tensor.matmul(
                                    aps[mi][:], w1_t[:, ko, :],
                                    xn[b][mi][:, ko, :],
                                    start=(ko == 0), stop=(ko == KD - 1))
                            for mi in range(NBT):
                                nc.tensor.matmul(
                                    bps[mi][:], w3_t[:, ko, :],
                                    xn[b][mi][:, ko, :],
                                    start=(ko == 0), stop=(ko == KD - 1))
                        ca_r = ca_in[b].rearrange("(nj ki) m -> ki nj m", ki=128)
                        for mi in range(NBT):
                            sil = sb.tile([128, MT], BF16, tag="sil", bufs=2,
                                          name=f"sil{l}_{b}_{nj}_{mi}")
                            nc.scalar.activation(sil[:], aps[mi][:], AF.Silu)
                            at = sb.tile([128, MT], BF16, tag="at", bufs=2,
                                         name=f"at{l}_{b}_{nj}_{mi}")
                            nc.vector.tensor_tensor(out=at[:], in0=bps[mi][:],
                                                    in1=sil[:], op=OP.mult)
                            nc.sync.dma_start(ca_r[:, nj, ts(mi, MT)], at[:])
                    if nj == 5:
                        co = dd.tile([HID_PAD, S], BF16, tag=f"ca_out{b}",
                                     bufs=2, addr_space="Shared",
                                     name=f"ca_out{l}_{b}")
                        nc.gpsimd.collective_compute(
                            "AllGather", OP.bypass, replica_groups=RG,
                            ins=[ca_in[b][:].opt()], outs=[co[:].opt()])
                        ca_out.append(co)

                # ---- J: w2 GEMM per batch + residual ----
                new_h = [sb.tile([128, 2, S], F32, tag=f"h2_{b}", bufs=1,
                                 name=f"h2_{l}_{b}") for b in range(B)]
                for b in range(B):
                    w2ps = [ps.tile([128, MT], F32, tag="acc", bufs=4,
                                    name=f"w2ps{l}_{b}_{i}") for i in range(4)]
                    ca_r = ca_out[b].rearrange("(ko ki) m -> ki ko m", ki=128)
                    for kb in range(W2KD // 4):
                        w2_t = sb.tile([128, 4, 256], BF16, tag="wk2", bufs=3,
                                       name=f"w2_{l}_{b}_{kb}")
                        nc.sync.dma_start(w2_t[:],
                                          w2_d[l, :, kb * 4:(kb + 1) * 4, :])
                        for j in range(4):
                            ko = kb * 4 + j
                            at2 = sb.tile([128, S], BF16, tag="kst", bufs=3,
                                          name=f"at2_{l}_{b}_{ko}")
                            nc.sync.dma_start(at2[:], ca_r[:, ko, :])
                            for njj in range(2):
                                for mi in range(NBT):
                                    nc.tensor.matmul(
                                        w2ps[mi * 2 + njj][:],
                                        w2_t[:, j, njj * 128:(njj + 1) * 128],
                                        at2[:, ts(mi, MT)],
                                        start=(ko == 0), stop=(ko == W2KD - 1))
                    ci = dd.tile([257, S], BF16, tag=f"ch2_in{b}", bufs=2,
                                 name=f"ch2_in{l}_{b}")
                    emit_resid(l, b, "2", new_h[b], w2ps, h1_sb[b], ci)
                    co = dd.tile([NCORES * 257, S], BF16, tag=f"ch2_out{b}",
                                 bufs=2, addr_space="Shared",
                                 name=f"ch2_out{l}_{b}")
                    nc.gpsimd.collective_compute(
                        "AllGather", OP.bypass, replica_groups=RG,
                        ins=[ci[:].opt()], outs=[co[:].opt()])
                    # next layer's norm1 (or the final norm) for this batch
                    norm_from_ag(b, co, f"K{l}")
                cur_h = new_h

            # ============ output head ============
            for nf in range(NF):
                wt = sb.tile([128, KD, 128], BF16, tag="wn", bufs=3,
                             name=f"wout{nf}")
                nc.sync.dma_start(wt[:], wout_d[:, nf])
                hps = [ps.tile([128, MT], F32, tag="acc", bufs=4,
                               name=f"hps{nf}_{i}") for i in range(4)]
                for ko in range(KD):
                    for gmi in range(4):
                        nc.tensor.matmul(
                            hps[gmi][:], wt[:, ko, :],
                            xn[gmi // 2][gmi % 2][:, ko, :],
                            start=(ko == 0), stop=(ko == KD - 1))
                for gmi in range(4):
                    lg = sb.tile([128, MT], F32, tag="lg", bufs=1,
                                 name=f"lg{nf}_{gmi}")
                    nc.scalar.copy(lg[:], hps[gmi][:])
                    nc.sync.dma_start(
                        logT_d[nf * 128:(nf + 1) * 128, ts(gmi, MT)], lg[:])

    nc.compile()
    return nc


_ROPE_PERM = np.concatenate([np.arange(0, HD, 2), np.arange(1, HD, 2)])


def _perm_heads(w):
    """Permute rope pairs within each 64-col head block. w: [K, n_heads*64]."""
    K, N = w.shape
    return np.ascontiguousarray(
        w.reshape(K, N // HD, HD)[:, :, _ROPE_PERM].reshape(K, N))


def _pack_k(w):
    """[K, N] -> [128, K//128, N] with feature f = ko*128 + ki."""
    K, N = w.shape
    return np.ascontiguousarray(w.reshape(K // 128, 128, N).transpose(1, 0, 2))


def _pack_n(w, nblk=128):
    """[K, N] -> [128, N//nblk, K//128, nblk]."""
    K, N = w.shape
    x = w.reshape(K // 128, 128, N // nblk, nblk)
    return np.ascontiguousarray(x.transpose(1, 2, 0, 3))


def _prep_inputs(inputs):
    f32 = np.float32
    tokens = np.asarray(inputs["tokens"]).astype(np.int64).reshape(-1)
    emb = np.asarray(inputs["emb_W"], dtype=f32)
    wq = np.asarray(inputs["wq"], dtype=f32)
    wk = np.asarray(inputs["wk"], dtype=f32)
    wv = np.asarray(inputs["wv"], dtype=f32)
    wo = np.asarray(inputs["wo"], dtype=f32)
    w1 = np.asarray(inputs["w1"], dtype=f32)
    w2 = np.asarray(inputs["w2"], dtype=f32)
    w3 = np.asarray(inputs["w3"], dtype=f32)
    an = np.asarray(inputs["attn_norm_w"], dtype=f32)
    fn = np.asarray(inputs["ffn_norm_w"], dtype=f32)
    nw = np.asarray(inputs["norm_w"], dtype=f32)
    outw = np.asarray(inputs["out_W"], dtype=f32)
    cos = np.asarray(inputs["freqs_cos"], dtype=f32)
    sin = np.asarray(inputs["freqs_sin"], dtype=f32)

    h0T = np.ascontiguousarray(emb[tokens].T)          # [2048, 2048] f32
    hn0 = _pack_k(h0T).astype(nbf16)                   # [128, 16, 2048]

    ct = np.ascontiguousarray(cos.T).astype(f32)       # [32, 1024]
    st = np.ascontiguousarray(sin.T).astype(f32)
    scale = np.float32(1.0 / np.sqrt(HD))
    cq, sq = scale * ct, scale * st
    TA = np.concatenate([cq, cq, cq, cq], axis=0).astype(nbf16)
    TBt = np.concatenate([sq, sq, sq, sq], axis=0).astype(nbf16)
    TKt = np.concatenate([ct, ct, st, st], axis=0).astype(nbf16)
    ropetab = np.stack([TA, TBt, TKt])                 # [3, 128, 1024]

    ssq0 = (h0T.astype(np.float64) ** 2).sum(axis=0).astype(f32)[None, :]

    tri = np.tril(np.ones((128, 128), dtype=bool))
    maskA = np.where(tri, 0.0, -1e30).astype(f32)

    wq_f = wq * an[:, :, None]
    wk_f = wk * an[:, :, None]
    wv_f = wv * an[:, :, None]
    w1_f = w1 * fn[:, :, None]
    w3_f = w3 * fn[:, :, None]
    outw_f = outw * nw[:, None]

    in_maps = []
    for r in range(NCORES):
        m = {
            "hn0": hn0,
            "h0r": np.ascontiguousarray(
                h0T[r * 256:(r + 1) * 256].reshape(2, 128, M).transpose(1, 0, 2)),
            "ropetab": ropetab,
            "maskA": maskA,
            "ssq0": ssq0,
        }
        wq_l, wkv_l, wo_l, w1_l, w3_l, w2_l = [], [], [], [], [], []
        for l in range(L):
            wq_r = _perm_heads(wq_f[l][:, r * 256:(r + 1) * 256])
            wq_l.append(_pack_n(wq_r.astype(nbf16)))
            wk_r = _perm_heads(wk_f[l][:, r * 64:(r + 1) * 64])
            wv_r = wv_f[l][:, r * 64:(r + 1) * 64]
            wkv_l.append(_pack_n(
                np.concatenate([wk_r, wv_r], axis=1).astype(nbf16))[:, 0])
            wo_l.append(_pack_k(
                wo[l][:, r * 256:(r + 1) * 256].astype(nbf16)))
            w1_r = np.zeros((DIM, FFN_PAD), dtype=f32)
            w1_r[:, :FFN_N] = w1_f[l][:, r * FFN_N:(r + 1) * FFN_N]
            w1_l.append(_pack_n(w1_r.astype(nbf16)))
            w3_r = np.zeros((DIM, FFN_PAD), dtype=f32)
            w3_r[:, :FFN_N] = w3_f[l][:, r * FFN_N:(r + 1) * FFN_N]
            w3_l.append(_pack_n(w3_r.astype(nbf16)))
            w2_r = np.zeros((HID_PAD, 256), dtype=f32)
            for jr in range(NCORES):
                w2_r[jr * FFN_PAD:jr * FFN_PAD + FFN_N] = \
                    w2[l][jr * FFN_N:(jr + 1) * FFN_N, r * 256:(r + 1) * 256]
            w2_l.append(_pack_k(w2_r.astype(nbf16)))
        m["wq"] = np.stack(wq_l)
        m["wkv"] = np.stack(wkv_l)
        m["wo"] = np.stack(wo_l)
        m["w1"] = np.stack(w1_l)
        m["w3"] = np.stack(w3_l)
        m["w2"] = np.stack(w2_l)
        wout_r = np.zeros((DIM, OUT_PAD), dtype=f32)
        wout_r[:, :OUT_N] = outw_f[:, r * OUT_N:(r + 1) * OUT_N]
        m["wout"] = _pack_n(wout_r.astype(nbf16))
        in_maps.append(m)
    return in_maps


_NC_CACHE = {}


def _get_nc():
    if "nc" not in _NC_CACHE:
        _NC_CACHE["nc"] = _build()
    return _NC_CACHE["nc"]


def run(inputs, trace=False):
    nc = _get_nc()
    in_maps = _prep_inputs(inputs)
    res = run_bass_kernel_spmd(nc, in_maps, core_ids=list(range(NCORES)),
                               trace=trace)
    logits = np.empty((M, VOCAB_TOTAL), dtype=np.float32)
    for r in range(NCORES):
        lt = res.results[r]["logT"]
        logits[:, r * OUT_N:(r + 1) * OUT_N] = lt[:OUT_N].T
    return logits.reshape(B, S, VOCAB_TOTAL), res


def kernel(**inputs):
    out, _ = run(inputs, trace=False)
    return out
